# revision 27
# baseline (speedup 1.0000x reference)
"""Trainium2 Bass kernel for GQA attention (B=2, S=2048, D=2048, H=16, G=4 kv-heads,
DH=128) with interleaved RoPE (base 1e6) and causal mask.

Sharding: one (batch b, kv-group g) pair per NeuronCore -> 8 cores. Each core
computes its 4 q-heads against its single kv-head (Megatron-style column-split
of w_q/w_k/w_v, row-split of w_o) and produces a partial (S, D) output-projection
product in bf16; the host sums the 4 partials per batch and adds bo.

Device dataflow per core (all matmuls bf16 with f32 PSUM accumulate):
  A) qT/kT/vT = W^T-slices @ x^T (transposed projections, dmodel contraction),
     interleaved RoPE applied in the transposed layout via a +-1 permutation
     matmul plus two DVE multiplies with host-provided cos/sin tables;
     v transposed back to [sk, dh] via PE transpose. Inputs arrive via a
     handful of batched rearranged DMAs (xT in 4 groups that the kk-outer
     kv-projection loop streams behind).
  B) per (head, sq-chunk of 512): scoresT tiles [sk=128, sq=512] via PE with a
     depth-2 software pipeline, exp on ScalarE (scale=1/sqrt(128)) straight out
     of PSUM -> bf16 attn weights, causal masking on diagonal tiles, PV matmul
     accumulates out^T [dh, sq] in PSUM over sk tiles. Softmax denominators
     accumulate on DVE as a shallow bf16 ladder tree (2x mode); the partition
     reduction AND broadcast happen in a single PE matmul against a 128x128
     ones matrix, inverted with the fast DVE reciprocal approximation (emitted
     after the fill work so the in-order DVE stream never bubbles), then a
     fused normalize multiply.
  C) partial = out_heads^T^T @ wo^T-slice; PSUM evictions alternate between
     ScalarE and DVE (both are loaded inside attention chunks), batched bf16
     DMAs out.
"""
import sys
import os

if '/opt/trn_rl_repo' not in sys.path:
    sys.path.insert(0, '/opt/trn_rl_repo')

import numpy as np
import ml_dtypes

from contextlib import ExitStack

import concourse.bass as bass
import concourse.bass_isa as bass_isa
import concourse.mybir as mybir
import concourse.tile as tile
from concourse import bacc
import concourse.bass_utils as bass_utils
from concourse.masks import make_identity

BF = mybir.dt.bfloat16
F32 = mybir.dt.float32
AF = mybir.ActivationFunctionType
ALU = mybir.AluOpType
RED = bass_isa.ReduceOp

B, S, D, H, G = 2, 2048, 2048, 16, 4
DH = 128
HPC = H // G          # q heads per core
KT = D // 128         # dmodel k-tiles
NCH = S // 512        # sq chunks
SCALE = float(1.0 / np.sqrt(DH))
N_CORES = 8

TRACE = False          # set by test harness to capture an NTFF profile
LAST_RESULTS = None    # BassKernelResults of the most recent run (for test.py)

_PROGRAM = None


def _build_program():
    nc = bacc.Bacc("TRN2", target_bir_lowering=False, debug=False,
                   num_devices=N_CORES)

    def din(name, shape, dtype=BF):
        return nc.dram_tensor(name, shape, dtype, kind="ExternalInput").ap()

    xT_d = din("xT", [D, S])
    wq_d = din("wqT", [D, 512])
    wk_d = din("wkT", [D, DH])
    wv_d = din("wvT", [D, DH])
    wo_d = din("woT", [512, D])
    cos_d = din("cosT", [DH, S])
    sin_d = din("sinT", [DH, S])
    perm_d = din("permT", [DH, DH])
    mask_d = din("maskLT", [DH, DH])
    bq_d = din("bq", [DH, HPC], F32)
    bk_d = din("bk", [DH, 1], F32)
    bv_d = din("bv", [DH, 1], F32)
    out_d = nc.dram_tensor("part", [S, D], BF, kind="ExternalOutput").ap()

    with tile.TileContext(nc) as tc, ExitStack() as ctx:
        consts = ctx.enter_context(tc.tile_pool(name="consts", bufs=1))
        # PSUM: poolM rotates 6 banks among kv/q-proj, rot, vtrans, the
        # depth-2 score pipeline and out-proj psums; poolO rotates 2 among
        # the ramp v-proj tails and the attention out accumulators.
        poolM = ctx.enter_context(tc.tile_pool(name="poolM", bufs=6, space="PSUM"))
        poolO = ctx.enter_context(tc.tile_pool(name="poolO", bufs=2, space="PSUM"))
        rawp = ctx.enter_context(tc.tile_pool(name="rawp", bufs=2))
        tmpp = ctx.enter_context(tc.tile_pool(name="tmpp", bufs=2))
        expp = ctx.enter_context(tc.tile_pool(name="expp", bufs=9))
        ladp = ctx.enter_context(tc.tile_pool(name="ladp", bufs=9))
        sigp = ctx.enter_context(tc.tile_pool(name="sigp", bufs=2))
        osbp = ctx.enter_context(tc.tile_pool(name="osbp", bufs=3))
        outup = ctx.enter_context(tc.tile_pool(name="outup", bufs=3))

        # persistent SBUF tensors
        wq_sb = consts.tile([128, KT, 512], BF, tag="wq")
        wk_sb = consts.tile([128, KT, DH], BF, tag="wk")
        wv_sb = consts.tile([128, KT, DH], BF, tag="wv")
        wo_sb = consts.tile([128, HPC, D], BF, tag="wo")
        mask_sb = consts.tile([128, 128], BF, tag="mask")
        bq_sb = consts.tile([128, HPC], F32, tag="bq")
        bk_sb = consts.tile([128, 1], F32, tag="bk")
        bv_sb = consts.tile([128, 1], F32, tag="bv")
        ones_sb = consts.tile([128, 128], BF, tag="ones")
        qT_sb = [consts.tile([128, S], BF, tag=f"qT{h}", name=f"qT{h}")
                 for h in range(HPC)]
        kT_sb = consts.tile([128, S], BF, tag="kT")
        v_sb = consts.tile([128, KT, DH], BF, tag="v")
        outT_sb = [consts.tile([128, HPC, 512], BF, tag=f"outT{c}", name=f"outT{c}")
                   for c in range(NCH)]
        xT_sb = consts.tile([128, KT, S], BF, tag="xT")
        cos_sb = consts.tile([128, S], BF, tag="cos")
        sin_sb = consts.tile([128, S], BF, tag="sin")
        perm_sb = consts.tile([128, 128], BF, tag="perm")
        ident_sb = consts.tile([128, 128], F32, tag="ident")

        # ---- input DMAs: 2D-sliced (3D strided views explode the
        # sequencer's descriptor-gen cost), consumption order, spread over
        # three triggering sequencers. wk/wv + small consts first so the
        # kk-outer kv-proj loop starts the moment the first xT tiles land;
        # wq/cos/sin/wo follow (needed progressively later).
        _dma_engines = [nc.sync, nc.gpsimd, nc.scalar]
        _dma_i = [0]

        def dma_in(out, in_):
            eng = _dma_engines[_dma_i[0] % len(_dma_engines)]
            _dma_i[0] += 1
            eng.dma_start(out=out, in_=in_)

        dma_in(wk_sb[:, 0, :], wk_d[0:128, :])
        dma_in(wv_sb[:, 0, :], wv_d[0:128, :])
        for kk in range(KT):
            dma_in(xT_sb[:, kk, :], xT_d[kk * 128:(kk + 1) * 128, :])
            if kk + 1 < KT:
                dma_in(wk_sb[:, kk + 1, :],
                       wk_d[(kk + 1) * 128:(kk + 2) * 128, :])
                dma_in(wv_sb[:, kk + 1, :],
                       wv_d[(kk + 1) * 128:(kk + 2) * 128, :])
            if kk == 0:
                dma_in(perm_sb, perm_d)
                dma_in(mask_sb, mask_d)
                dma_in(bq_sb, bq_d)
                dma_in(bk_sb, bk_d)
                dma_in(bv_sb, bv_d)
        for kk in range(KT):
            dma_in(wq_sb[:, kk, :], wq_d[kk * 128:(kk + 1) * 128, :])
        dma_in(cos_sb, cos_d)
        dma_in(sin_sb, sin_d)
        for h in range(HPC):
            dma_in(wo_sb[:, h, :], wo_d[h * 128:(h + 1) * 128, :])
        nc.vector.memset(ones_sb, 1.0)
        make_identity(nc, ident_sb)

        def rope_store(raw, dst, bias_ap, c):
            rot = poolM.tile([128, 512], F32, tag="m512", name="rot")
            nc.tensor.matmul(rot, perm_sb, raw, start=True, stop=True)
            t1 = tmpp.tile([128, 512], BF, tag="t1", name="t1")
            nc.vector.tensor_mul(t1, raw, cos_sb[:, c * 512:(c + 1) * 512])
            t2 = tmpp.tile([128, 512], BF, tag="t2", name="t2")
            nc.vector.tensor_mul(t2, rot, sin_sb[:, c * 512:(c + 1) * 512])
            # dst = (t2 + bias) + t1
            nc.vector.scalar_tensor_tensor(dst, t2, bias_ap, t1,
                                           op0=ALU.add, op1=ALU.add)

        # ---- ramp: k and v projections together, kk-outer (8 concurrent
        # PSUM accumulators) so PE density tracks the xT DMA stream.
        pss_k = [poolM.tile([128, 512], F32, tag="m512", name=f"kps{_c}")
                 for _c in range(NCH)]
        pss_v = [poolM.tile([128, 512], F32, tag="m512", name=f"vps{_c}")
                 for _c in range(2)]
        pss_v += [poolO.tile([128, 512], F32, tag="o512", name=f"vps{_c}")
                  for _c in range(2, NCH)]
        for kk in range(KT):
            for c in range(NCH):
                nc.tensor.matmul(pss_k[c], wk_sb[:, kk, :],
                                 xT_sb[:, kk, c * 512:(c + 1) * 512],
                                 start=(kk == 0), stop=(kk == KT - 1))
            for c in range(NCH):
                nc.tensor.matmul(pss_v[c], wv_sb[:, kk, :],
                                 xT_sb[:, kk, c * 512:(c + 1) * 512],
                                 start=(kk == 0), stop=(kk == KT - 1))
        for c in range(NCH):
            raw = rawp.tile([128, 512], BF, tag="kraw", name="kraw")
            nc.scalar.copy(raw, pss_k[c])
            vraw = rawp.tile([128, 512], F32, tag="vraw", name="vraw")
            nc.scalar.activation(vraw, pss_v[c], func=AF.Identity,
                                 bias=bv_sb[:, 0:1])
            rope_store(raw, kT_sb[:, c * 512:(c + 1) * 512], bk_sb[:, 0:1], c)
            for j in range(4):
                t = c * 4 + j
                tp = poolM.tile([128, 128], F32, tag="m512", name="vtps")
                nc.tensor.transpose(tp, vraw[:, j * 128:(j + 1) * 128],
                                    ident_sb)
                nc.vector.tensor_copy(v_sb[:, t, :], tp)

        def emit_q_mms(h):
            pss = [poolM.tile([128, 512], F32, tag="m512", name=f"qps{_c}")
                   for _c in range(NCH)]
            for kk in range(KT):
                for c in range(NCH):
                    nc.tensor.matmul(pss[c],
                                     wq_sb[:, kk, h * 128:(h + 1) * 128],
                                     xT_sb[:, kk, c * 512:(c + 1) * 512],
                                     start=(kk == 0), stop=(kk == KT - 1))
            raws = []
            for c in range(NCH):
                raw = rawp.tile([128, 512], BF, tag="qraw", name="qraw",
                                bufs=8)
                nc.scalar.copy(raw, pss[c])
                raws.append(raw)
            return raws

        def emit_q_rope(h, raws):
            for c in range(NCH):
                rope_store(raws[c], qT_sb[h][:, c * 512:(c + 1) * 512],
                           bq_sb[:, h:h + 1], c)

        def emit_q_proj(h):
            emit_q_rope(h, emit_q_mms(h))

        def emit_c_group(m, np_, dve=True, ns=None, both_dve=False):
            if ns is None:
                ns = (2 * np_, 2 * np_ + 1)
            mc, mo = divmod(m, 4)
            pso = {n: poolM.tile([128, 512], F32, tag="m512", name=f"cpsum{n}")
                   for n in ns}
            for h in range(HPC):
                for n in ns:
                    nc.tensor.matmul(pso[n],
                                     outT_sb[mc][:, h, mo * 128:(mo + 1) * 128],
                                     wo_sb[:, h, n * 512:(n + 1) * 512],
                                     start=(h == 0), stop=(h == HPC - 1))
            ob = osbp.tile([128, 512 * len(ns)], BF, tag="osb", name="osb")
            for j, n in enumerate(ns):
                # split the PSUM evictions across both PSUM-capable engines
                if both_dve or (j % 2 == 0) == dve:
                    nc.vector.tensor_copy(ob[:, j * 512:(j + 1) * 512], pso[n])
                else:
                    nc.scalar.copy(ob[:, j * 512:(j + 1) * 512], pso[n])
            nc.gpsimd.dma_start(
                out=out_d[m * 128:(m + 1) * 128,
                          ns[0] * 512:(ns[-1] + 1) * 512],
                in_=ob)

        def emit_b_pair(c, hp, fill_ms=()):
            nt = 4 * c + 4
            hs = (2 * hp, 2 * hp + 1)
            out_ps = {h: poolO.tile([128, 512], F32, tag="o512",
                                    name=f"outps{h}") for h in hs}
            # bf16 ladder tree for the softmax denominators: quad partial
            # sums (depth <=3) combined pairwise at the end; stays in DVE's
            # 2x 16-bit mode and keeps rounding depth ~5 (<0.5% on sigma).
            lad = {h: [] for h in hs}
            cur = {h: None for h in hs}

            def emit_scores(t):
                jb = t - 4 * c
                off = max(jb, 0) * 128   # first valid sq column
                cl, ch_ = c * 512 + off, (c + 1) * 512
                es = {}
                for h in hs:
                    s_ps = poolM.tile([128, 512], F32, tag="m512", name="s_ps")
                    nc.tensor.matmul(s_ps[:, off:],
                                     kT_sb[:, t * 128:(t + 1) * 128],
                                     qT_sb[h][:, cl:ch_],
                                     start=True, stop=True)
                    e = expp.tile([128, 512], BF, tag="exp", name="e")
                    nc.scalar.activation(e[:, off:], s_ps[:, off:],
                                         func=AF.Exp, scale=SCALE)
                    if jb >= 0:
                        nc.vector.tensor_mul(e[:, off:off + 128],
                                             e[:, off:off + 128], mask_sb)
                    es[h] = e
                return es

            def emit_consume(t, es):
                jb = t - 4 * c
                off = max(jb, 0) * 128
                for h in hs:
                    nc.tensor.matmul(out_ps[h][:, off:], v_sb[:, t, :],
                                     es[h][:, off:],
                                     start=(t == 0), stop=(t == nt - 1))
                for h in hs:
                    e = es[h]
                    if jb < 0:          # full-width tile, quad position t%4
                        if t % 4 == 0:
                            q = ladp.tile([128, 512], BF, tag="lad",
                                          name="lad")
                            cur[h] = q
                            nc.vector.tensor_copy(q, e)
                        else:
                            nc.vector.tensor_add(cur[h], cur[h], e)
                            if t % 4 == 3:
                                lad[h].append(cur[h])
                                cur[h] = None
                    else:               # diagonal group (off grows with jb)
                        if jb == 0:
                            q = ladp.tile([128, 512], BF, tag="lad",
                                          name="lad")
                            cur[h] = q
                            nc.vector.tensor_copy(q, e)
                        else:
                            nc.vector.tensor_add(cur[h][:, off:],
                                                 cur[h][:, off:], e[:, off:])
                        if jb == 3:
                            lad[h].append(cur[h])
                            cur[h] = None

            # depth-2 software pipeline: scores for t+1 and t+2 are in
            # flight before the PV/ladder consumers of t, so PE never waits
            # on exp. Fill out-proj groups are spread at even t positions so
            # PE has slack work exactly when exp pacing would stall it.
            fq = [(m, np_) for m in fill_ms for np_ in range(NCH // 2)]
            fill_at = {max(2, round((i + 1) * nt / (len(fq) + 1))): i
                       for i in range(len(fq))} if fq else {}
            pend = []
            for t in range(min(2, nt)):
                pend.append(emit_scores(t))
            for t in range(2, nt):
                emit_consume(t - 2, pend.pop(0))
                pend.append(emit_scores(t))
                if t in fill_at:
                    m, np_ = fq[fill_at[t]]
                    emit_c_group(m, np_, dve=(c >= 2), both_dve=(c >= 2))
            for i, es in enumerate(pend):
                emit_consume(nt - len(pend) + i, es)
            for t, i in fill_at.items():
                if t >= nt:
                    m, np_ = fq[i]
                    emit_c_group(m, np_, dve=(c >= 2), both_dve=(c >= 2))

            # pairwise-combine the quad sums into one sigma ladder tile
            for h in hs:
                ts = lad[h]
                while len(ts) > 1:
                    nxt = []
                    for i in range(0, len(ts) - 1, 2):
                        nc.vector.tensor_add(ts[i], ts[i], ts[i + 1])
                        nxt.append(ts[i])
                    if len(ts) % 2:
                        nxt.append(ts[-1])
                    ts = nxt
                lad[h] = ts[0]

            # evict accumulators to SBUF with fast ACT copies so the PSUM
            # banks free quickly
            outU = {}
            for h in hs:
                u = outup.tile([128, 512], F32, tag="outU", name="outU")
                if c >= 2:
                    nc.vector.tensor_copy(u, out_ps[h])
                else:
                    nc.scalar.copy(u, out_ps[h])
                outU[h] = u


            # sigma reduce+broadcast in one PE matmul against the ones
            # matrix (out[p,n] = sum_k lad[k,n] for every p), then the fast
            # reciprocal approximation and the fused normalize multiply.
            # Emitted after the fills so the in-order DVE stream never
            # bubbles waiting on cross-engine chains; outT[c] is only
            # needed a chunk boundary later.
            sgp = {}
            for h in hs:
                sg = poolO.tile([128, 512], F32, tag="o512", name="sgps")
                nc.tensor.matmul(sg, ones_sb, lad[h], start=True, stop=True)
                sgp[h] = sg
            for h in hs:
                rinv = sigp.tile([128, 512], F32, tag="sig", name="rinv")
                nc.vector.reciprocal_approx_fast(rinv, sgp[h])
                nc.vector.tensor_mul(outT_sb[c][:, h, :], outU[h], rinv)

        # ---- interleave: q-projections sandwich the first attention
        # chunk; each head's rope emits after the NEXT head's projection
        # block so the rot matmuls never wait on the ACT eviction.
        r0 = emit_q_mms(0)
        r1 = emit_q_mms(1)
        emit_q_rope(0, r0)
        emit_q_rope(1, r1)
        emit_b_pair(0, 0)
        r2 = emit_q_mms(2)
        r3 = emit_q_mms(3)
        emit_q_rope(2, r2)
        emit_q_rope(3, r3)
        emit_b_pair(0, 1)
        for c in range(1, NCH):
            for hp in range(HPC // 2):
                base = 4 * (c - 1) + 2 * hp
                emit_b_pair(c, hp, fill_ms=(base, base + 1))
        for m in range(4 * (NCH - 1), 4 * NCH):
            emit_c_group(m, 0, dve=(m % 2 == 1), ns=(0, 1))
            emit_c_group(m, 1, dve=(m % 2 == 0), ns=(2, 3))

    nc.compile()
    return nc


def _get_program():
    global _PROGRAM
    if _PROGRAM is None:
        _PROGRAM = _build_program()
    return _PROGRAM


def _host_tables():
    bf16 = ml_dtypes.bfloat16
    pos = np.arange(S, dtype=np.float32)[:, None]
    i = np.arange(DH // 2, dtype=np.float32)
    omega = np.exp((-2.0 * i / DH * np.log(np.float32(1_000_000.0))).astype(np.float32))
    ang = (pos * omega).astype(np.float32)
    sinT = np.ascontiguousarray(np.repeat(np.sin(ang), 2, axis=-1).T)
    cosT = np.ascontiguousarray(np.repeat(np.cos(ang), 2, axis=-1).T)
    P = np.zeros((DH, DH), np.float32)
    for ii in range(DH // 2):
        P[2 * ii, 2 * ii + 1] = -1.0
        P[2 * ii + 1, 2 * ii] = 1.0
    permT = np.ascontiguousarray(P.T).astype(bf16)
    maskLT = np.triu(np.ones((128, 128), np.float32)).astype(bf16)
    return cosT, sinT, permT, maskLT


def _install_ntff_hook():
    """Optional: register the axon NTFF profiling hook (missing antenv.axon_hooks
    shim) so run_bass_kernel_spmd(trace=True) can capture HW exec time."""
    import types
    try:
        import antenv
        if 'antenv.axon_hooks' not in sys.modules:
            mod = types.ModuleType('antenv.axon_hooks')
            _hook = [None]
            mod.set_axon_ntff_profile_hook = lambda h: _hook.__setitem__(0, h)
            mod.get_axon_ntff_profile_hook = lambda: _hook[0]
            sys.modules['antenv.axon_hooks'] = mod
            antenv.axon_hooks = mod
        if '/root/.axon_site' not in sys.path:
            sys.path.insert(0, '/root/.axon_site')
        from trn_agent_boot.trn_boot import _ntff_profile_via_ctypes
        sys.modules['antenv.axon_hooks'].set_axon_ntff_profile_hook(
            _ntff_profile_via_ctypes('/opt/axon/libaxon_pjrt.so'))
        bass_utils.upload_artifacts = lambda tmpdir: tmpdir
        return True
    except Exception:
        return False


def kernel(x, wq, bq, wk, bk, wv, bv, wo, bo, masked=None, **_unused):
    global LAST_RESULTS
    bf16 = ml_dtypes.bfloat16
    nc = _get_program()

    x = np.asarray(x, np.float32)
    wq = np.asarray(wq, np.float32)
    wk = np.asarray(wk, np.float32)
    wv = np.asarray(wv, np.float32)
    wo = np.asarray(wo, np.float32)
    bq = np.asarray(bq, np.float32)
    bk = np.asarray(bk, np.float32)
    bv = np.asarray(bv, np.float32)
    bo = np.asarray(bo, np.float32)

    cosT, sinT, permT, maskLT = _host_tables()

    xT = [np.ascontiguousarray(x[b].T).astype(bf16) for b in range(B)]
    in_maps = []
    for core in range(N_CORES):
        b, g = divmod(core, G)
        cs = slice(g * 512, (g + 1) * 512)          # q-channel / out-channel slice
        ks = slice(g * 128, (g + 1) * 128)          # kv-channel slice
        in_maps.append({
            "xT": xT[b],
            "wqT": np.ascontiguousarray(wq[cs, :].T).astype(bf16),
            "wkT": np.ascontiguousarray(wk[ks, :].T).astype(bf16),
            "wvT": np.ascontiguousarray(wv[ks, :].T).astype(bf16),
            "woT": np.ascontiguousarray(wo[:, cs].T).astype(bf16),
            "cosT": cosT.astype(bf16),
            "sinT": sinT.astype(bf16),
            "permT": permT,
            "maskLT": maskLT,
            "bq": np.ascontiguousarray(bq[cs].reshape(HPC, DH).T),
            "bk": np.ascontiguousarray(bk[ks].reshape(DH, 1)),
            "bv": np.ascontiguousarray(bv[ks].reshape(DH, 1)),
        })

    trace = bool(TRACE)
    if trace:
        trace = _install_ntff_hook()
    res = bass_utils.run_bass_kernel_spmd(nc, in_maps,
                                          core_ids=list(range(N_CORES)),
                                          trace=trace)
    LAST_RESULTS = res

    out = np.zeros((B, S, D), np.float32)
    for core in range(N_CORES):
        b = core // G
        out[b] += np.asarray(res.results[core]["part"], np.float32)
    out += bo[None, None, :]
    return out


# revision 28
# speedup vs baseline: 1.1311x; 1.1311x over previous
"""Trainium2 Bass kernel for GQA attention (B=2, S=2048, D=2048, H=16, G=4 kv-heads,
DH=128) with interleaved RoPE (base 1e6) and causal mask.

Sharding: one (batch b, kv-group g) pair per NeuronCore -> 8 cores. Each core
computes its 4 q-heads against its single kv-head (Megatron-style column-split
of w_q/w_k/w_v, row-split of w_o) and produces a partial (S, D) output-projection
product in bf16; the host sums the 4 partials per batch and adds bo.

Device dataflow per core (all matmuls bf16 with f32 PSUM accumulate):
  A) qT/kT/vT = W^T-slices @ x^T (transposed projections, dmodel contraction),
     interleaved RoPE applied in the transposed layout via a +-1 permutation
     matmul plus two DVE multiplies with host-provided cos/sin tables;
     v transposed back to [sk, dh] via PE transpose. Inputs arrive via a
     handful of batched rearranged DMAs (xT in 4 groups that the kk-outer
     kv-projection loop streams behind).
  B) per (head, sq-chunk of 512): scoresT tiles [sk=128, sq=512] via PE with a
     depth-2 software pipeline, exp on ScalarE (scale=1/sqrt(128)) straight out
     of PSUM -> bf16 attn weights, causal masking on diagonal tiles, PV matmul
     accumulates out^T [dh, sq] in PSUM over sk tiles. Softmax denominators
     accumulate on DVE as a shallow bf16 ladder tree (2x mode); the partition
     reduction AND broadcast happen in a single PE matmul against a 128x128
     ones matrix, inverted with the fast DVE reciprocal approximation (emitted
     after the fill work so the in-order DVE stream never bubbles), then a
     fused normalize multiply.
  C) partial = out_heads^T^T @ wo^T-slice; PSUM evictions alternate between
     ScalarE and DVE (both are loaded inside attention chunks), batched bf16
     DMAs out.
"""
import sys
import os

if '/opt/trn_rl_repo' not in sys.path:
    sys.path.insert(0, '/opt/trn_rl_repo')

import numpy as np
import ml_dtypes

from contextlib import ExitStack

import concourse.bass as bass
import concourse.bass_isa as bass_isa
import concourse.mybir as mybir
import concourse.tile as tile
from concourse import bacc
import concourse.bass_utils as bass_utils
from concourse.masks import make_identity

BF = mybir.dt.bfloat16
F32 = mybir.dt.float32
AF = mybir.ActivationFunctionType
ALU = mybir.AluOpType
RED = bass_isa.ReduceOp

B, S, D, H, G = 2, 2048, 2048, 16, 4
DH = 128
HPC = H // G          # q heads per core
KT = D // 128         # dmodel k-tiles
NCH = S // 512        # sq chunks
SCALE = float(1.0 / np.sqrt(DH))
N_CORES = 8

TRACE = False          # set by test harness to capture an NTFF profile
LAST_RESULTS = None    # BassKernelResults of the most recent run (for test.py)

_PROGRAM = None


def _build_program():
    nc = bacc.Bacc("TRN2", target_bir_lowering=False, debug=False,
                   num_devices=N_CORES)

    def din(name, shape, dtype=BF):
        return nc.dram_tensor(name, shape, dtype, kind="ExternalInput").ap()

    xT_d = din("xT", [D, S])
    wq_d = din("wqT", [D, 512])
    wk_d = din("wkT", [D, DH])
    wv_d = din("wvT", [D, DH])
    wo_d = din("woT", [512, D])
    cos_d = din("cosT", [DH, S])
    sin_d = din("sinT", [DH, S])
    perm_d = din("permT", [DH, DH])
    mask_d = din("maskLT", [DH, DH])
    bq_d = din("bq", [DH, HPC], F32)
    bk_d = din("bk", [DH, 1], F32)
    bv_d = din("bv", [DH, 1], F32)
    out_d = nc.dram_tensor("part", [S, D], BF, kind="ExternalOutput").ap()

    with tile.TileContext(nc) as tc, ExitStack() as ctx:
        consts = ctx.enter_context(tc.tile_pool(name="consts", bufs=1))
        # PSUM: poolM rotates 6 banks among kv/q-proj, rot, vtrans, the
        # depth-2 score pipeline and out-proj psums; poolO rotates 2 among
        # the ramp v-proj tails and the attention out accumulators.
        poolM = ctx.enter_context(tc.tile_pool(name="poolM", bufs=6, space="PSUM"))
        poolO = ctx.enter_context(tc.tile_pool(name="poolO", bufs=2, space="PSUM"))
        rawp = ctx.enter_context(tc.tile_pool(name="rawp", bufs=2))
        tmpp = ctx.enter_context(tc.tile_pool(name="tmpp", bufs=2))
        expp = ctx.enter_context(tc.tile_pool(name="expp", bufs=9))
        ladp = ctx.enter_context(tc.tile_pool(name="ladp", bufs=9))
        sigp = ctx.enter_context(tc.tile_pool(name="sigp", bufs=2))
        osbp = ctx.enter_context(tc.tile_pool(name="osbp", bufs=3))
        outup = ctx.enter_context(tc.tile_pool(name="outup", bufs=3))

        # persistent SBUF tensors
        wq_sb = consts.tile([128, KT, 512], BF, tag="wq")
        wk_sb = consts.tile([128, KT, DH], BF, tag="wk")
        wv_sb = consts.tile([128, KT, DH], BF, tag="wv")
        wo_sb = consts.tile([128, HPC, D], BF, tag="wo")
        mask_sb = consts.tile([128, 128], BF, tag="mask")
        bq_sb = consts.tile([128, HPC], F32, tag="bq")
        bk_sb = consts.tile([128, 1], F32, tag="bk")
        bv_sb = consts.tile([128, 1], F32, tag="bv")
        ones_sb = consts.tile([128, 128], BF, tag="ones")
        qT_sb = [consts.tile([128, S], BF, tag=f"qT{h}", name=f"qT{h}")
                 for h in range(HPC)]
        kT_sb = consts.tile([128, S], BF, tag="kT")
        v_sb = consts.tile([128, KT, DH], BF, tag="v")
        outT_sb = [consts.tile([128, HPC, 512], BF, tag=f"outT{c}", name=f"outT{c}")
                   for c in range(NCH)]
        xT_sb = consts.tile([128, KT, S], BF, tag="xT")
        cos_sb = consts.tile([128, S], BF, tag="cos")
        sin_sb = consts.tile([128, S], BF, tag="sin")
        perm_sb = consts.tile([128, 128], BF, tag="perm")
        ident_sb = consts.tile([128, 128], F32, tag="ident")

        # ---- input DMAs: 2D-sliced (3D strided views explode the
        # sequencer's descriptor-gen cost), consumption order, spread over
        # three triggering sequencers. wk/wv + small consts first so the
        # kk-outer kv-proj loop starts the moment the first xT tiles land;
        # wq/cos/sin/wo follow (needed progressively later).
        _dma_engines = [nc.sync, nc.gpsimd, nc.scalar]
        _dma_i = [0]

        def dma_in(out, in_):
            eng = _dma_engines[_dma_i[0] % len(_dma_engines)]
            _dma_i[0] += 1
            eng.dma_start(out=out, in_=in_)

        dma_in(wk_sb[:, 0, :], wk_d[0:128, :])
        dma_in(wv_sb[:, 0, :], wv_d[0:128, :])
        for kk in range(KT):
            dma_in(xT_sb[:, kk, :], xT_d[kk * 128:(kk + 1) * 128, :])
            if kk + 1 < KT:
                dma_in(wk_sb[:, kk + 1, :],
                       wk_d[(kk + 1) * 128:(kk + 2) * 128, :])
                dma_in(wv_sb[:, kk + 1, :],
                       wv_d[(kk + 1) * 128:(kk + 2) * 128, :])
            if kk == 0:
                dma_in(perm_sb, perm_d)
                dma_in(mask_sb, mask_d)
                dma_in(bq_sb, bq_d)
                dma_in(bk_sb, bk_d)
                dma_in(bv_sb, bv_d)
        for kk in range(KT):
            dma_in(wq_sb[:, kk, :], wq_d[kk * 128:(kk + 1) * 128, :])
        dma_in(cos_sb, cos_d)
        dma_in(sin_sb, sin_d)
        for h in range(HPC):
            dma_in(wo_sb[:, h, :], wo_d[h * 128:(h + 1) * 128, :])
        nc.vector.memset(ones_sb, 1.0)
        make_identity(nc, ident_sb)

        def rope_store(raw, dst, bias_ap, c):
            rot = poolM.tile([128, 512], F32, tag="m512", name="rot")
            nc.tensor.matmul(rot, perm_sb, raw, start=True, stop=True)
            t1 = tmpp.tile([128, 512], BF, tag="t1", name="t1")
            nc.vector.tensor_mul(t1, raw, cos_sb[:, c * 512:(c + 1) * 512])
            t2 = tmpp.tile([128, 512], BF, tag="t2", name="t2")
            nc.vector.tensor_mul(t2, rot, sin_sb[:, c * 512:(c + 1) * 512])
            # dst = (t2 + bias) + t1
            nc.vector.scalar_tensor_tensor(dst, t2, bias_ap, t1,
                                           op0=ALU.add, op1=ALU.add)

        # ---- ramp: k and v projections together, kk-outer (8 concurrent
        # PSUM accumulators) so PE density tracks the xT DMA stream.
        pss_k = [poolM.tile([128, 512], F32, tag="m512", name=f"kps{_c}")
                 for _c in range(NCH)]
        pss_v = [poolM.tile([128, 512], F32, tag="m512", name=f"vps{_c}")
                 for _c in range(2)]
        pss_v += [poolO.tile([128, 512], F32, tag="o512", name=f"vps{_c}")
                  for _c in range(2, NCH)]
        for kk in range(KT):
            for c in range(NCH):
                nc.tensor.matmul(pss_k[c], wk_sb[:, kk, :],
                                 xT_sb[:, kk, c * 512:(c + 1) * 512],
                                 start=(kk == 0), stop=(kk == KT - 1))
            for c in range(NCH):
                nc.tensor.matmul(pss_v[c], wv_sb[:, kk, :],
                                 xT_sb[:, kk, c * 512:(c + 1) * 512],
                                 start=(kk == 0), stop=(kk == KT - 1))
        for c in range(NCH):
            raw = rawp.tile([128, 512], BF, tag="kraw", name="kraw")
            nc.scalar.copy(raw, pss_k[c])
            rope_store(raw, kT_sb[:, c * 512:(c + 1) * 512], bk_sb[:, 0:1], c)
            vraw = rawp.tile([128, 512], F32, tag="vraw", name="vraw")
            nc.scalar.activation(vraw, pss_v[c], func=AF.Identity,
                                 bias=bv_sb[:, 0:1])
            for j in range(4):
                t = c * 4 + j
                tp = poolM.tile([128, 128], F32, tag="m512", name="vtps")
                nc.tensor.transpose(tp, vraw[:, j * 128:(j + 1) * 128],
                                    ident_sb)
                nc.vector.tensor_copy(v_sb[:, t, :], tp)

        def emit_q_mms(h):
            pss = [poolM.tile([128, 512], F32, tag="m512", name=f"qps{_c}")
                   for _c in range(NCH)]
            for kk in range(KT):
                for c in range(NCH):
                    nc.tensor.matmul(pss[c],
                                     wq_sb[:, kk, h * 128:(h + 1) * 128],
                                     xT_sb[:, kk, c * 512:(c + 1) * 512],
                                     start=(kk == 0), stop=(kk == KT - 1))
            raws = []
            for c in range(NCH):
                raw = rawp.tile([128, 512], BF, tag="qraw", name="qraw",
                                bufs=8)
                nc.scalar.copy(raw, pss[c])
                raws.append(raw)
            return raws

        def emit_q_rope(h, raws):
            for c in range(NCH):
                rope_store(raws[c], qT_sb[h][:, c * 512:(c + 1) * 512],
                           bq_sb[:, h:h + 1], c)

        def emit_q_proj(h):
            emit_q_rope(h, emit_q_mms(h))

        def emit_c_group(m, np_, dve=True, ns=None, both_dve=False):
            if ns is None:
                ns = (2 * np_, 2 * np_ + 1)
            mc, mo = divmod(m, 4)
            pso = {n: poolM.tile([128, 512], F32, tag="m512", name=f"cpsum{n}")
                   for n in ns}
            for h in range(HPC):
                for n in ns:
                    nc.tensor.matmul(pso[n],
                                     outT_sb[mc][:, h, mo * 128:(mo + 1) * 128],
                                     wo_sb[:, h, n * 512:(n + 1) * 512],
                                     start=(h == 0), stop=(h == HPC - 1))
            ob = osbp.tile([128, 512 * len(ns)], BF, tag="osb", name="osb")
            for j, n in enumerate(ns):
                # split the PSUM evictions across both PSUM-capable engines
                if both_dve or (j % 2 == 0) == dve:
                    nc.vector.tensor_copy(ob[:, j * 512:(j + 1) * 512], pso[n])
                else:
                    nc.scalar.copy(ob[:, j * 512:(j + 1) * 512], pso[n])
            nc.gpsimd.dma_start(
                out=out_d[m * 128:(m + 1) * 128,
                          ns[0] * 512:(ns[-1] + 1) * 512],
                in_=ob)

        def emit_b_pair(c, hp, fill_ms=()):
            nt = 4 * c + 4
            hs = (2 * hp, 2 * hp + 1)
            out_ps = {h: poolO.tile([128, 512], F32, tag="o512",
                                    name=f"outps{h}") for h in hs}
            # bf16 ladder tree for the softmax denominators: quad partial
            # sums (depth <=3) combined pairwise at the end; stays in DVE's
            # 2x 16-bit mode and keeps rounding depth ~5 (<0.5% on sigma).
            lad = {h: [] for h in hs}
            cur = {h: None for h in hs}

            def emit_scores(t):
                jb = t - 4 * c
                off = max(jb, 0) * 128   # first valid sq column
                cl, ch_ = c * 512 + off, (c + 1) * 512
                es = {}
                for h in hs:
                    s_ps = poolM.tile([128, 512], F32, tag="m512", name="s_ps")
                    nc.tensor.matmul(s_ps[:, off:],
                                     kT_sb[:, t * 128:(t + 1) * 128],
                                     qT_sb[h][:, cl:ch_],
                                     start=True, stop=True)
                    e = expp.tile([128, 512], BF, tag="exp", name="e")
                    nc.scalar.activation(e[:, off:], s_ps[:, off:],
                                         func=AF.Exp, scale=SCALE)
                    if jb >= 0:
                        nc.vector.tensor_mul(e[:, off:off + 128],
                                             e[:, off:off + 128], mask_sb)
                    es[h] = e
                return es

            def emit_consume(t, es):
                jb = t - 4 * c
                off = max(jb, 0) * 128
                for h in hs:
                    nc.tensor.matmul(out_ps[h][:, off:], v_sb[:, t, :],
                                     es[h][:, off:],
                                     start=(t == 0), stop=(t == nt - 1))
                for h in hs:
                    e = es[h]
                    if jb < 0:          # full-width tile, quad position t%4
                        if t % 4 == 0:
                            q = ladp.tile([128, 512], BF, tag="lad",
                                          name="lad")
                            cur[h] = q
                            nc.vector.tensor_copy(q, e)
                        else:
                            nc.vector.tensor_add(cur[h], cur[h], e)
                            if t % 4 == 3:
                                lad[h].append(cur[h])
                                cur[h] = None
                    else:               # diagonal group (off grows with jb)
                        if jb == 0:
                            q = ladp.tile([128, 512], BF, tag="lad",
                                          name="lad")
                            cur[h] = q
                            nc.vector.tensor_copy(q, e)
                        else:
                            nc.vector.tensor_add(cur[h][:, off:],
                                                 cur[h][:, off:], e[:, off:])
                        if jb == 3:
                            lad[h].append(cur[h])
                            cur[h] = None

            # depth-2 software pipeline: scores for t+1 and t+2 are in flight
            # before the PV/ladder consumers of t, so PE never waits on exp
            pend = []
            for t in range(min(2, nt)):
                pend.append(emit_scores(t))
            for t in range(2, nt):
                emit_consume(t - 2, pend.pop(0))
                pend.append(emit_scores(t))
            for i, es in enumerate(pend):
                emit_consume(nt - len(pend) + i, es)

            # pairwise-combine the quad sums into one sigma ladder tile
            for h in hs:
                ts = lad[h]
                while len(ts) > 1:
                    nxt = []
                    for i in range(0, len(ts) - 1, 2):
                        nc.vector.tensor_add(ts[i], ts[i], ts[i + 1])
                        nxt.append(ts[i])
                    if len(ts) % 2:
                        nxt.append(ts[-1])
                    ts = nxt
                lad[h] = ts[0]

            # evict accumulators to SBUF with fast ACT copies so the PSUM
            # banks free quickly
            outU = {}
            for h in hs:
                u = outup.tile([128, 512], F32, tag="outU", name="outU")
                if c >= 2:
                    nc.vector.tensor_copy(u, out_ps[h])
                else:
                    nc.scalar.copy(u, out_ps[h])
                outU[h] = u

            # fill the boundary chain latency with out-proj work of the
            # previous chunk (its outT rows are complete)
            for m in fill_ms:
                for np_ in range(NCH // 2):
                    emit_c_group(m, np_, dve=(c >= 2), both_dve=(c >= 2))

            # sigma reduce+broadcast in one PE matmul against the ones
            # matrix (out[p,n] = sum_k lad[k,n] for every p), then the fast
            # reciprocal approximation and the fused normalize multiply.
            # Emitted after the fills so the in-order DVE stream never
            # bubbles waiting on cross-engine chains; outT[c] is only
            # needed a chunk boundary later.
            sgp = {}
            for h in hs:
                sg = poolO.tile([128, 512], F32, tag="o512", name="sgps")
                nc.tensor.matmul(sg, ones_sb, lad[h], start=True, stop=True)
                sgp[h] = sg
            for h in hs:
                rinv = sigp.tile([128, 512], F32, tag="sig", name="rinv")
                nc.vector.reciprocal_approx_fast(rinv, sgp[h])
                nc.vector.tensor_mul(outT_sb[c][:, h, :], outU[h], rinv)

        # ---- interleave: q-projections sandwich the first attention
        # chunk; each head's rope emits after the NEXT head's projection
        # block so the rot matmuls never wait on the ACT eviction.
        r0 = emit_q_mms(0)
        r1 = emit_q_mms(1)
        emit_q_rope(0, r0)
        emit_q_rope(1, r1)
        emit_b_pair(0, 0)
        r2 = emit_q_mms(2)
        r3 = emit_q_mms(3)
        emit_q_rope(2, r2)
        emit_q_rope(3, r3)
        emit_b_pair(0, 1)
        for c in range(1, NCH):
            for hp in range(HPC // 2):
                base = 4 * (c - 1) + 2 * hp
                emit_b_pair(c, hp, fill_ms=(base, base + 1))
        for m in range(4 * (NCH - 1), 4 * NCH):
            emit_c_group(m, 0, dve=(m % 2 == 1), ns=(0, 1))
            emit_c_group(m, 1, dve=(m % 2 == 0), ns=(2, 3))

    nc.compile()
    return nc


def _get_program():
    global _PROGRAM
    if _PROGRAM is None:
        _PROGRAM = _build_program()
    return _PROGRAM


def _host_tables():
    bf16 = ml_dtypes.bfloat16
    pos = np.arange(S, dtype=np.float32)[:, None]
    i = np.arange(DH // 2, dtype=np.float32)
    omega = np.exp((-2.0 * i / DH * np.log(np.float32(1_000_000.0))).astype(np.float32))
    ang = (pos * omega).astype(np.float32)
    sinT = np.ascontiguousarray(np.repeat(np.sin(ang), 2, axis=-1).T)
    cosT = np.ascontiguousarray(np.repeat(np.cos(ang), 2, axis=-1).T)
    P = np.zeros((DH, DH), np.float32)
    for ii in range(DH // 2):
        P[2 * ii, 2 * ii + 1] = -1.0
        P[2 * ii + 1, 2 * ii] = 1.0
    permT = np.ascontiguousarray(P.T).astype(bf16)
    maskLT = np.triu(np.ones((128, 128), np.float32)).astype(bf16)
    return cosT, sinT, permT, maskLT


def _install_ntff_hook():
    """Optional: register the axon NTFF profiling hook (missing antenv.axon_hooks
    shim) so run_bass_kernel_spmd(trace=True) can capture HW exec time."""
    import types
    try:
        import antenv
        if 'antenv.axon_hooks' not in sys.modules:
            mod = types.ModuleType('antenv.axon_hooks')
            _hook = [None]
            mod.set_axon_ntff_profile_hook = lambda h: _hook.__setitem__(0, h)
            mod.get_axon_ntff_profile_hook = lambda: _hook[0]
            sys.modules['antenv.axon_hooks'] = mod
            antenv.axon_hooks = mod
        if '/root/.axon_site' not in sys.path:
            sys.path.insert(0, '/root/.axon_site')
        from trn_agent_boot.trn_boot import _ntff_profile_via_ctypes
        sys.modules['antenv.axon_hooks'].set_axon_ntff_profile_hook(
            _ntff_profile_via_ctypes('/opt/axon/libaxon_pjrt.so'))
        bass_utils.upload_artifacts = lambda tmpdir: tmpdir
        return True
    except Exception:
        return False


def kernel(x, wq, bq, wk, bk, wv, bv, wo, bo, masked=None, **_unused):
    global LAST_RESULTS
    bf16 = ml_dtypes.bfloat16
    nc = _get_program()

    x = np.asarray(x, np.float32)
    wq = np.asarray(wq, np.float32)
    wk = np.asarray(wk, np.float32)
    wv = np.asarray(wv, np.float32)
    wo = np.asarray(wo, np.float32)
    bq = np.asarray(bq, np.float32)
    bk = np.asarray(bk, np.float32)
    bv = np.asarray(bv, np.float32)
    bo = np.asarray(bo, np.float32)

    cosT, sinT, permT, maskLT = _host_tables()

    xT = [np.ascontiguousarray(x[b].T).astype(bf16) for b in range(B)]
    in_maps = []
    for core in range(N_CORES):
        b, g = divmod(core, G)
        cs = slice(g * 512, (g + 1) * 512)          # q-channel / out-channel slice
        ks = slice(g * 128, (g + 1) * 128)          # kv-channel slice
        in_maps.append({
            "xT": xT[b],
            "wqT": np.ascontiguousarray(wq[cs, :].T).astype(bf16),
            "wkT": np.ascontiguousarray(wk[ks, :].T).astype(bf16),
            "wvT": np.ascontiguousarray(wv[ks, :].T).astype(bf16),
            "woT": np.ascontiguousarray(wo[:, cs].T).astype(bf16),
            "cosT": cosT.astype(bf16),
            "sinT": sinT.astype(bf16),
            "permT": permT,
            "maskLT": maskLT,
            "bq": np.ascontiguousarray(bq[cs].reshape(HPC, DH).T),
            "bk": np.ascontiguousarray(bk[ks].reshape(DH, 1)),
            "bv": np.ascontiguousarray(bv[ks].reshape(DH, 1)),
        })

    trace = bool(TRACE)
    if trace:
        trace = _install_ntff_hook()
    res = bass_utils.run_bass_kernel_spmd(nc, in_maps,
                                          core_ids=list(range(N_CORES)),
                                          trace=trace)
    LAST_RESULTS = res

    out = np.zeros((B, S, D), np.float32)
    for core in range(N_CORES):
        b = core // G
        out[b] += np.asarray(res.results[core]["part"], np.float32)
    out += bo[None, None, :]
    return out


# revision 29
# speedup vs baseline: 1.1385x; 1.0065x over previous
"""Trainium2 Bass kernel for GQA attention (B=2, S=2048, D=2048, H=16, G=4 kv-heads,
DH=128) with interleaved RoPE (base 1e6) and causal mask.

Sharding: one (batch b, kv-group g) pair per NeuronCore -> 8 cores. Each core
computes its 4 q-heads against its single kv-head (Megatron-style column-split
of w_q/w_k/w_v, row-split of w_o) and produces a partial (S, D) output-projection
product in bf16; the host sums the 4 partials per batch and adds bo.

Device dataflow per core (all matmuls bf16 with f32 PSUM accumulate):
  A) qT/kT/vT = W^T-slices @ x^T (transposed projections, dmodel contraction),
     interleaved RoPE applied in the transposed layout via a +-1 permutation
     matmul plus two DVE multiplies with host-provided cos/sin tables;
     v transposed back to [sk, dh] via PE transpose. Inputs arrive via a
     handful of batched rearranged DMAs (xT in 4 groups that the kk-outer
     kv-projection loop streams behind).
  B) per (head, sq-chunk of 512): scoresT tiles [sk=128, sq=512] via PE with a
     depth-2 software pipeline, exp on ScalarE (scale=1/sqrt(128)) straight out
     of PSUM -> bf16 attn weights, causal masking on diagonal tiles, PV matmul
     accumulates out^T [dh, sq] in PSUM over sk tiles. Softmax denominators
     accumulate on DVE as a shallow bf16 ladder tree (2x mode); the partition
     reduction AND broadcast happen in a single PE matmul against a 128x128
     ones matrix, inverted with the fast DVE reciprocal approximation (emitted
     after the fill work so the in-order DVE stream never bubbles), then a
     fused normalize multiply.
  C) partial = out_heads^T^T @ wo^T-slice; PSUM evictions alternate between
     ScalarE and DVE (both are loaded inside attention chunks), batched bf16
     DMAs out.
"""
import sys
import os

if '/opt/trn_rl_repo' not in sys.path:
    sys.path.insert(0, '/opt/trn_rl_repo')

import numpy as np
import ml_dtypes

from contextlib import ExitStack

import concourse.bass as bass
import concourse.bass_isa as bass_isa
import concourse.mybir as mybir
import concourse.tile as tile
from concourse import bacc
import concourse.bass_utils as bass_utils
from concourse.masks import make_identity

BF = mybir.dt.bfloat16
F32 = mybir.dt.float32
AF = mybir.ActivationFunctionType
ALU = mybir.AluOpType
RED = bass_isa.ReduceOp

B, S, D, H, G = 2, 2048, 2048, 16, 4
DH = 128
HPC = H // G          # q heads per core
KT = D // 128         # dmodel k-tiles
NCH = S // 512        # sq chunks
SCALE = float(1.0 / np.sqrt(DH))
N_CORES = 8

TRACE = False          # set by test harness to capture an NTFF profile
LAST_RESULTS = None    # BassKernelResults of the most recent run (for test.py)

_PROGRAM = None


def _build_program():
    nc = bacc.Bacc("TRN2", target_bir_lowering=False, debug=False,
                   num_devices=N_CORES)

    def din(name, shape, dtype=BF):
        return nc.dram_tensor(name, shape, dtype, kind="ExternalInput").ap()

    xT_d = din("xT", [D, S])
    wq_d = din("wqT", [D, 512])
    wk_d = din("wkT", [D, DH])
    wv_d = din("wvT", [D, DH])
    wo_d = din("woT", [512, D])
    cos_d = din("cosT", [DH, S])
    sin_d = din("sinT", [DH, S])
    perm_d = din("permT", [DH, DH])
    mask_d = din("maskLT", [DH, DH])
    bq_d = din("bq", [DH, HPC], F32)
    bk_d = din("bk", [DH, 1], F32)
    bv_d = din("bv", [DH, 1], F32)
    out_d = nc.dram_tensor("part", [S, D], BF, kind="ExternalOutput").ap()

    with tile.TileContext(nc) as tc, ExitStack() as ctx:
        consts = ctx.enter_context(tc.tile_pool(name="consts", bufs=1))
        # PSUM: poolM rotates 6 banks among kv/q-proj, rot, vtrans, the
        # depth-2 score pipeline and out-proj psums; poolO rotates 2 among
        # the ramp v-proj tails and the attention out accumulators.
        poolM = ctx.enter_context(tc.tile_pool(name="poolM", bufs=6, space="PSUM"))
        poolO = ctx.enter_context(tc.tile_pool(name="poolO", bufs=2, space="PSUM"))
        rawp = ctx.enter_context(tc.tile_pool(name="rawp", bufs=2))
        tmpp = ctx.enter_context(tc.tile_pool(name="tmpp", bufs=2))
        expp = ctx.enter_context(tc.tile_pool(name="expp", bufs=9))
        ladp = ctx.enter_context(tc.tile_pool(name="ladp", bufs=9))
        sigp = ctx.enter_context(tc.tile_pool(name="sigp", bufs=2))
        osbp = ctx.enter_context(tc.tile_pool(name="osbp", bufs=3))
        outup = ctx.enter_context(tc.tile_pool(name="outup", bufs=3))

        # persistent SBUF tensors
        wq_sb = consts.tile([128, KT, 512], BF, tag="wq")
        wk_sb = consts.tile([128, KT, DH], BF, tag="wk")
        wv_sb = consts.tile([128, KT, DH], BF, tag="wv")
        wo_sb = consts.tile([128, HPC, D], BF, tag="wo")
        mask_sb = consts.tile([128, 128], BF, tag="mask")
        bq_sb = consts.tile([128, HPC], F32, tag="bq")
        bk_sb = consts.tile([128, 1], F32, tag="bk")
        bv_sb = consts.tile([128, 1], F32, tag="bv")
        ones_sb = consts.tile([128, 128], BF, tag="ones")
        qT_sb = [consts.tile([128, S], BF, tag=f"qT{h}", name=f"qT{h}")
                 for h in range(HPC)]
        kT_sb = consts.tile([128, S], BF, tag="kT")
        v_sb = consts.tile([128, KT, DH], BF, tag="v")
        outT_sb = [consts.tile([128, HPC, 512], BF, tag=f"outT{c}", name=f"outT{c}")
                   for c in range(NCH)]
        xT_sb = consts.tile([128, KT, S], BF, tag="xT")
        cos_sb = consts.tile([128, S], BF, tag="cos")
        sin_sb = consts.tile([128, S], BF, tag="sin")
        perm_sb = consts.tile([128, 128], BF, tag="perm")
        ident_sb = consts.tile([128, 128], F32, tag="ident")

        # ---- input DMAs: 2D-sliced (3D strided views explode the
        # sequencer's descriptor-gen cost), consumption order, spread over
        # three triggering sequencers. wk/wv + small consts first so the
        # kk-outer kv-proj loop starts the moment the first xT tiles land;
        # wq/cos/sin/wo follow (needed progressively later).
        _dma_engines = [nc.sync, nc.gpsimd, nc.scalar]
        _dma_i = [0]

        def dma_in(out, in_):
            eng = _dma_engines[_dma_i[0] % len(_dma_engines)]
            _dma_i[0] += 1
            eng.dma_start(out=out, in_=in_)

        dma_in(wk_sb[:, 0, :], wk_d[0:128, :])
        dma_in(wv_sb[:, 0, :], wv_d[0:128, :])
        for kk in range(KT):
            dma_in(xT_sb[:, kk, :], xT_d[kk * 128:(kk + 1) * 128, :])
            if kk + 1 < KT:
                dma_in(wk_sb[:, kk + 1, :],
                       wk_d[(kk + 1) * 128:(kk + 2) * 128, :])
                dma_in(wv_sb[:, kk + 1, :],
                       wv_d[(kk + 1) * 128:(kk + 2) * 128, :])
            if kk == 0:
                dma_in(perm_sb, perm_d)
                dma_in(mask_sb, mask_d)
                dma_in(bq_sb, bq_d)
                dma_in(bk_sb, bk_d)
                dma_in(bv_sb, bv_d)
        for kk in range(KT):
            dma_in(wq_sb[:, kk, :], wq_d[kk * 128:(kk + 1) * 128, :])
        dma_in(cos_sb, cos_d)
        dma_in(sin_sb, sin_d)
        for h in range(HPC):
            dma_in(wo_sb[:, h, :], wo_d[h * 128:(h + 1) * 128, :])
        nc.vector.memset(ones_sb, 1.0)
        make_identity(nc, ident_sb)

        def rope_store(raw, dst, bias_ap, c):
            # rot lives in poolO (never blocks the score-psum ring) and is
            # evicted to SBUF by ScalarE immediately, so the sin-multiply
            # runs in DVE's 2x bf16 mode instead of 1x f32-from-PSUM and
            # the bank frees right away.
            rot = poolO.tile([128, 512], F32, tag="o512", name="rot")
            nc.tensor.matmul(rot, perm_sb, raw, start=True, stop=True)
            rsb = tmpp.tile([128, 512], BF, tag="rsb", name="rsb")
            nc.scalar.copy(rsb, rot)
            t1 = tmpp.tile([128, 512], BF, tag="t1", name="t1")
            nc.vector.tensor_mul(t1, raw, cos_sb[:, c * 512:(c + 1) * 512])
            t2 = tmpp.tile([128, 512], BF, tag="t2", name="t2")
            nc.vector.tensor_mul(t2, rsb, sin_sb[:, c * 512:(c + 1) * 512])
            # dst = (t2 + bias) + t1
            nc.vector.scalar_tensor_tensor(dst, t2, bias_ap, t1,
                                           op0=ALU.add, op1=ALU.add)

        # ---- ramp: k and v projections together, kk-outer (8 concurrent
        # PSUM accumulators) so PE density tracks the xT DMA stream.
        pss_k = [poolM.tile([128, 512], F32, tag="m512", name=f"kps{_c}")
                 for _c in range(NCH)]
        pss_v = [poolM.tile([128, 512], F32, tag="m512", name=f"vps{_c}")
                 for _c in range(2)]
        pss_v += [poolO.tile([128, 512], F32, tag="o512", name=f"vps{_c}")
                  for _c in range(2, NCH)]
        for kk in range(KT):
            for c in range(NCH):
                nc.tensor.matmul(pss_k[c], wk_sb[:, kk, :],
                                 xT_sb[:, kk, c * 512:(c + 1) * 512],
                                 start=(kk == 0), stop=(kk == KT - 1))
            for c in range(NCH):
                nc.tensor.matmul(pss_v[c], wv_sb[:, kk, :],
                                 xT_sb[:, kk, c * 512:(c + 1) * 512],
                                 start=(kk == 0), stop=(kk == KT - 1))
        for c in range(NCH):
            raw = rawp.tile([128, 512], BF, tag="kraw", name="kraw")
            nc.scalar.copy(raw, pss_k[c])
            rope_store(raw, kT_sb[:, c * 512:(c + 1) * 512], bk_sb[:, 0:1], c)
            vraw = rawp.tile([128, 512], F32, tag="vraw", name="vraw")
            nc.scalar.activation(vraw, pss_v[c], func=AF.Identity,
                                 bias=bv_sb[:, 0:1])
            for j in range(4):
                t = c * 4 + j
                tp = poolM.tile([128, 128], F32, tag="m512", name="vtps")
                nc.tensor.transpose(tp, vraw[:, j * 128:(j + 1) * 128],
                                    ident_sb)
                nc.vector.tensor_copy(v_sb[:, t, :], tp)

        def emit_q_mms(h):
            pss = [poolM.tile([128, 512], F32, tag="m512", name=f"qps{_c}")
                   for _c in range(NCH)]
            for kk in range(KT):
                for c in range(NCH):
                    nc.tensor.matmul(pss[c],
                                     wq_sb[:, kk, h * 128:(h + 1) * 128],
                                     xT_sb[:, kk, c * 512:(c + 1) * 512],
                                     start=(kk == 0), stop=(kk == KT - 1))
            raws = []
            for c in range(NCH):
                raw = rawp.tile([128, 512], BF, tag="qraw", name="qraw",
                                bufs=8)
                nc.scalar.copy(raw, pss[c])
                raws.append(raw)
            return raws

        def emit_q_rope(h, raws):
            for c in range(NCH):
                rope_store(raws[c], qT_sb[h][:, c * 512:(c + 1) * 512],
                           bq_sb[:, h:h + 1], c)

        def emit_q_proj(h):
            emit_q_rope(h, emit_q_mms(h))

        def emit_c_group(m, np_, dve=True, ns=None, both_dve=False):
            if ns is None:
                ns = (2 * np_, 2 * np_ + 1)
            mc, mo = divmod(m, 4)
            pso = {n: poolM.tile([128, 512], F32, tag="m512", name=f"cpsum{n}")
                   for n in ns}
            for h in range(HPC):
                for n in ns:
                    nc.tensor.matmul(pso[n],
                                     outT_sb[mc][:, h, mo * 128:(mo + 1) * 128],
                                     wo_sb[:, h, n * 512:(n + 1) * 512],
                                     start=(h == 0), stop=(h == HPC - 1))
            ob = osbp.tile([128, 512 * len(ns)], BF, tag="osb", name="osb")
            for j, n in enumerate(ns):
                # split the PSUM evictions across both PSUM-capable engines
                if both_dve or (j % 2 == 0) == dve:
                    nc.vector.tensor_copy(ob[:, j * 512:(j + 1) * 512], pso[n])
                else:
                    nc.scalar.copy(ob[:, j * 512:(j + 1) * 512], pso[n])
            nc.gpsimd.dma_start(
                out=out_d[m * 128:(m + 1) * 128,
                          ns[0] * 512:(ns[-1] + 1) * 512],
                in_=ob)

        def emit_b_pair(c, hp, fill_ms=()):
            nt = 4 * c + 4
            hs = (2 * hp, 2 * hp + 1)
            out_ps = {h: poolO.tile([128, 512], F32, tag="o512",
                                    name=f"outps{h}") for h in hs}
            # bf16 ladder tree for the softmax denominators: quad partial
            # sums (depth <=3) combined pairwise at the end; stays in DVE's
            # 2x 16-bit mode and keeps rounding depth ~5 (<0.5% on sigma).
            lad = {h: [] for h in hs}
            cur = {h: None for h in hs}

            def emit_scores(t):
                jb = t - 4 * c
                off = max(jb, 0) * 128   # first valid sq column
                cl, ch_ = c * 512 + off, (c + 1) * 512
                es = {}
                for h in hs:
                    s_ps = poolM.tile([128, 512], F32, tag="m512", name="s_ps")
                    nc.tensor.matmul(s_ps[:, off:],
                                     kT_sb[:, t * 128:(t + 1) * 128],
                                     qT_sb[h][:, cl:ch_],
                                     start=True, stop=True)
                    e = expp.tile([128, 512], BF, tag="exp", name="e")
                    nc.scalar.activation(e[:, off:], s_ps[:, off:],
                                         func=AF.Exp, scale=SCALE)
                    if jb >= 0:
                        nc.vector.tensor_mul(e[:, off:off + 128],
                                             e[:, off:off + 128], mask_sb)
                    es[h] = e
                return es

            def emit_consume(t, es):
                jb = t - 4 * c
                off = max(jb, 0) * 128
                for h in hs:
                    nc.tensor.matmul(out_ps[h][:, off:], v_sb[:, t, :],
                                     es[h][:, off:],
                                     start=(t == 0), stop=(t == nt - 1))
                for h in hs:
                    e = es[h]
                    if jb < 0:          # full-width tile, quad position t%4
                        if t % 4 == 0:
                            q = ladp.tile([128, 512], BF, tag="lad",
                                          name="lad")
                            cur[h] = q
                            nc.vector.tensor_copy(q, e)
                        else:
                            nc.vector.tensor_add(cur[h], cur[h], e)
                            if t % 4 == 3:
                                lad[h].append(cur[h])
                                cur[h] = None
                    else:               # diagonal group (off grows with jb)
                        if jb == 0:
                            q = ladp.tile([128, 512], BF, tag="lad",
                                          name="lad")
                            cur[h] = q
                            nc.vector.tensor_copy(q, e)
                        else:
                            nc.vector.tensor_add(cur[h][:, off:],
                                                 cur[h][:, off:], e[:, off:])
                        if jb == 3:
                            lad[h].append(cur[h])
                            cur[h] = None

            # depth-2 software pipeline: scores for t+1 and t+2 are in flight
            # before the PV/ladder consumers of t, so PE never waits on exp
            pend = []
            for t in range(min(2, nt)):
                pend.append(emit_scores(t))
            for t in range(2, nt):
                emit_consume(t - 2, pend.pop(0))
                pend.append(emit_scores(t))
            for i, es in enumerate(pend):
                emit_consume(nt - len(pend) + i, es)

            # pairwise-combine the quad sums into one sigma ladder tile
            for h in hs:
                ts = lad[h]
                while len(ts) > 1:
                    nxt = []
                    for i in range(0, len(ts) - 1, 2):
                        nc.vector.tensor_add(ts[i], ts[i], ts[i + 1])
                        nxt.append(ts[i])
                    if len(ts) % 2:
                        nxt.append(ts[-1])
                    ts = nxt
                lad[h] = ts[0]

            # evict accumulators to SBUF with fast ACT copies so the PSUM
            # banks free quickly
            outU = {}
            for h in hs:
                u = outup.tile([128, 512], F32, tag="outU", name="outU")
                if c >= 2:
                    nc.vector.tensor_copy(u, out_ps[h])
                else:
                    nc.scalar.copy(u, out_ps[h])
                outU[h] = u

            # fill the boundary chain latency with out-proj work of the
            # previous chunk (its outT rows are complete)
            for m in fill_ms:
                for np_ in range(NCH // 2):
                    emit_c_group(m, np_, dve=(c >= 2), both_dve=(c >= 2))

            # sigma reduce+broadcast in one PE matmul against the ones
            # matrix (out[p,n] = sum_k lad[k,n] for every p), then the fast
            # reciprocal approximation and the fused normalize multiply.
            # Emitted after the fills so the in-order DVE stream never
            # bubbles waiting on cross-engine chains; outT[c] is only
            # needed a chunk boundary later.
            sgp = {}
            for h in hs:
                sg = poolO.tile([128, 512], F32, tag="o512", name="sgps")
                nc.tensor.matmul(sg, ones_sb, lad[h], start=True, stop=True)
                sgp[h] = sg
            for h in hs:
                rinv = sigp.tile([128, 512], F32, tag="sig", name="rinv")
                nc.vector.reciprocal_approx_fast(rinv, sgp[h])
                nc.vector.tensor_mul(outT_sb[c][:, h, :], outU[h], rinv)

        # ---- interleave: q-projections sandwich the first attention
        # chunk; each head's rope emits after the NEXT head's projection
        # block so the rot matmuls never wait on the ACT eviction.
        r0 = emit_q_mms(0)
        r1 = emit_q_mms(1)
        emit_q_rope(0, r0)
        emit_q_rope(1, r1)
        emit_b_pair(0, 0)
        r2 = emit_q_mms(2)
        r3 = emit_q_mms(3)
        emit_q_rope(2, r2)
        emit_q_rope(3, r3)
        emit_b_pair(0, 1)
        for c in range(1, NCH):
            for hp in range(HPC // 2):
                base = 4 * (c - 1) + 2 * hp
                emit_b_pair(c, hp, fill_ms=(base, base + 1))
        for m in range(4 * (NCH - 1), 4 * NCH):
            emit_c_group(m, 0, dve=(m % 2 == 1), ns=(0, 1))
            emit_c_group(m, 1, dve=(m % 2 == 0), ns=(2, 3))

    nc.compile()
    return nc


def _get_program():
    global _PROGRAM
    if _PROGRAM is None:
        _PROGRAM = _build_program()
    return _PROGRAM


def _host_tables():
    bf16 = ml_dtypes.bfloat16
    pos = np.arange(S, dtype=np.float32)[:, None]
    i = np.arange(DH // 2, dtype=np.float32)
    omega = np.exp((-2.0 * i / DH * np.log(np.float32(1_000_000.0))).astype(np.float32))
    ang = (pos * omega).astype(np.float32)
    sinT = np.ascontiguousarray(np.repeat(np.sin(ang), 2, axis=-1).T)
    cosT = np.ascontiguousarray(np.repeat(np.cos(ang), 2, axis=-1).T)
    P = np.zeros((DH, DH), np.float32)
    for ii in range(DH // 2):
        P[2 * ii, 2 * ii + 1] = -1.0
        P[2 * ii + 1, 2 * ii] = 1.0
    permT = np.ascontiguousarray(P.T).astype(bf16)
    maskLT = np.triu(np.ones((128, 128), np.float32)).astype(bf16)
    return cosT, sinT, permT, maskLT


def _install_ntff_hook():
    """Optional: register the axon NTFF profiling hook (missing antenv.axon_hooks
    shim) so run_bass_kernel_spmd(trace=True) can capture HW exec time."""
    import types
    try:
        import antenv
        if 'antenv.axon_hooks' not in sys.modules:
            mod = types.ModuleType('antenv.axon_hooks')
            _hook = [None]
            mod.set_axon_ntff_profile_hook = lambda h: _hook.__setitem__(0, h)
            mod.get_axon_ntff_profile_hook = lambda: _hook[0]
            sys.modules['antenv.axon_hooks'] = mod
            antenv.axon_hooks = mod
        if '/root/.axon_site' not in sys.path:
            sys.path.insert(0, '/root/.axon_site')
        from trn_agent_boot.trn_boot import _ntff_profile_via_ctypes
        sys.modules['antenv.axon_hooks'].set_axon_ntff_profile_hook(
            _ntff_profile_via_ctypes('/opt/axon/libaxon_pjrt.so'))
        bass_utils.upload_artifacts = lambda tmpdir: tmpdir
        return True
    except Exception:
        return False


def kernel(x, wq, bq, wk, bk, wv, bv, wo, bo, masked=None, **_unused):
    global LAST_RESULTS
    bf16 = ml_dtypes.bfloat16
    nc = _get_program()

    x = np.asarray(x, np.float32)
    wq = np.asarray(wq, np.float32)
    wk = np.asarray(wk, np.float32)
    wv = np.asarray(wv, np.float32)
    wo = np.asarray(wo, np.float32)
    bq = np.asarray(bq, np.float32)
    bk = np.asarray(bk, np.float32)
    bv = np.asarray(bv, np.float32)
    bo = np.asarray(bo, np.float32)

    cosT, sinT, permT, maskLT = _host_tables()

    xT = [np.ascontiguousarray(x[b].T).astype(bf16) for b in range(B)]
    in_maps = []
    for core in range(N_CORES):
        b, g = divmod(core, G)
        cs = slice(g * 512, (g + 1) * 512)          # q-channel / out-channel slice
        ks = slice(g * 128, (g + 1) * 128)          # kv-channel slice
        in_maps.append({
            "xT": xT[b],
            "wqT": np.ascontiguousarray(wq[cs, :].T).astype(bf16),
            "wkT": np.ascontiguousarray(wk[ks, :].T).astype(bf16),
            "wvT": np.ascontiguousarray(wv[ks, :].T).astype(bf16),
            "woT": np.ascontiguousarray(wo[:, cs].T).astype(bf16),
            "cosT": cosT.astype(bf16),
            "sinT": sinT.astype(bf16),
            "permT": permT,
            "maskLT": maskLT,
            "bq": np.ascontiguousarray(bq[cs].reshape(HPC, DH).T),
            "bk": np.ascontiguousarray(bk[ks].reshape(DH, 1)),
            "bv": np.ascontiguousarray(bv[ks].reshape(DH, 1)),
        })

    trace = bool(TRACE)
    if trace:
        trace = _install_ntff_hook()
    res = bass_utils.run_bass_kernel_spmd(nc, in_maps,
                                          core_ids=list(range(N_CORES)),
                                          trace=trace)
    LAST_RESULTS = res

    out = np.zeros((B, S, D), np.float32)
    for core in range(N_CORES):
        b = core // G
        out[b] += np.asarray(res.results[core]["part"], np.float32)
    out += bo[None, None, :]
    return out


# revision 31
# speedup vs baseline: 1.1515x; 1.0114x over previous
"""Trainium2 Bass kernel for GQA attention (B=2, S=2048, D=2048, H=16, G=4 kv-heads,
DH=128) with interleaved RoPE (base 1e6) and causal mask.

Sharding: one (batch b, kv-group g) pair per NeuronCore -> 8 cores. Each core
computes its 4 q-heads against its single kv-head (Megatron-style column-split
of w_q/w_k/w_v, row-split of w_o) and produces a partial (S, D) output-projection
product in bf16; the host sums the 4 partials per batch and adds bo.

Device dataflow per core (all matmuls bf16 with f32 PSUM accumulate):
  A) qT/kT/vT = W^T-slices @ x^T (transposed projections, dmodel contraction),
     interleaved RoPE applied in the transposed layout via a +-1 permutation
     matmul plus two DVE multiplies with host-provided cos/sin tables;
     v transposed back to [sk, dh] via PE transpose. Inputs arrive via a
     handful of batched rearranged DMAs (xT in 4 groups that the kk-outer
     kv-projection loop streams behind).
  B) per (head, sq-chunk of 512): scoresT tiles [sk=128, sq=512] via PE with a
     depth-2 software pipeline, exp on ScalarE (scale=1/sqrt(128)) straight out
     of PSUM -> bf16 attn weights, causal masking on diagonal tiles, PV matmul
     accumulates out^T [dh, sq] in PSUM over sk tiles. Softmax denominators
     accumulate on DVE as a shallow bf16 ladder tree (2x mode); the partition
     reduction AND broadcast happen in a single PE matmul against a 128x128
     ones matrix, inverted with the fast DVE reciprocal approximation (emitted
     after the fill work so the in-order DVE stream never bubbles), then a
     fused normalize multiply.
  C) partial = out_heads^T^T @ wo^T-slice; PSUM evictions alternate between
     ScalarE and DVE (both are loaded inside attention chunks), batched bf16
     DMAs out.
"""
import sys
import os

if '/opt/trn_rl_repo' not in sys.path:
    sys.path.insert(0, '/opt/trn_rl_repo')

import numpy as np
import ml_dtypes

from contextlib import ExitStack

import concourse.bass as bass
import concourse.bass_isa as bass_isa
import concourse.mybir as mybir
import concourse.tile as tile
from concourse import bacc
import concourse.bass_utils as bass_utils
from concourse.masks import make_identity

BF = mybir.dt.bfloat16
F32 = mybir.dt.float32
AF = mybir.ActivationFunctionType
ALU = mybir.AluOpType
RED = bass_isa.ReduceOp

B, S, D, H, G = 2, 2048, 2048, 16, 4
DH = 128
HPC = H // G          # q heads per core
KT = D // 128         # dmodel k-tiles
NCH = S // 512        # sq chunks
SCALE = float(1.0 / np.sqrt(DH))
N_CORES = 8

TRACE = False          # set by test harness to capture an NTFF profile
LAST_RESULTS = None    # BassKernelResults of the most recent run (for test.py)

_PROGRAM = None


def _build_program():
    nc = bacc.Bacc("TRN2", target_bir_lowering=False, debug=False,
                   num_devices=N_CORES)

    def din(name, shape, dtype=BF):
        return nc.dram_tensor(name, shape, dtype, kind="ExternalInput").ap()

    xT_d = din("xT", [D, S])
    wq_d = din("wqT", [D, 512])
    wk_d = din("wkT", [D, DH])
    wv_d = din("wvT", [D, DH])
    wo_d = din("woT", [512, D])
    cos_d = din("cosT", [DH, S])
    sin_d = din("sinT", [DH, S])
    perm_d = din("permT", [DH, DH])
    mask_d = din("maskLT", [DH, DH])
    bq_d = din("bq", [DH, HPC], F32)
    bk_d = din("bk", [DH, 1], F32)
    bv_d = din("bv", [DH, 1], F32)
    out_d = nc.dram_tensor("part", [S, D], BF, kind="ExternalOutput").ap()

    with tile.TileContext(nc) as tc, ExitStack() as ctx:
        consts = ctx.enter_context(tc.tile_pool(name="consts", bufs=1))
        # PSUM: poolM rotates 6 banks among kv/q-proj, rot, vtrans, the
        # depth-2 score pipeline and out-proj psums; poolO rotates 2 among
        # the ramp v-proj tails and the attention out accumulators.
        poolM = ctx.enter_context(tc.tile_pool(name="poolM", bufs=6, space="PSUM"))
        poolO = ctx.enter_context(tc.tile_pool(name="poolO", bufs=2, space="PSUM"))
        rawp = ctx.enter_context(tc.tile_pool(name="rawp", bufs=2))
        tmpp = ctx.enter_context(tc.tile_pool(name="tmpp", bufs=2))
        expp = ctx.enter_context(tc.tile_pool(name="expp", bufs=9))
        ladp = ctx.enter_context(tc.tile_pool(name="ladp", bufs=9))
        sigp = ctx.enter_context(tc.tile_pool(name="sigp", bufs=2))
        osbp = ctx.enter_context(tc.tile_pool(name="osbp", bufs=3))
        outup = ctx.enter_context(tc.tile_pool(name="outup", bufs=3))

        # persistent SBUF tensors
        wq_sb = consts.tile([128, KT, 512], BF, tag="wq")
        wk_sb = consts.tile([128, KT, DH], BF, tag="wk")
        wv_sb = consts.tile([128, KT, DH], BF, tag="wv")
        wo_sb = consts.tile([128, HPC, D], BF, tag="wo")
        mask_sb = consts.tile([128, 128], BF, tag="mask")
        bq_sb = consts.tile([128, HPC], F32, tag="bq")
        bk_sb = consts.tile([128, 1], F32, tag="bk")
        bv_sb = consts.tile([128, 1], F32, tag="bv")
        ones_sb = consts.tile([128, 128], BF, tag="ones")
        qT_sb = [consts.tile([128, S], BF, tag=f"qT{h}", name=f"qT{h}")
                 for h in range(HPC)]
        kT_sb = consts.tile([128, S], BF, tag="kT")
        v_sb = consts.tile([128, KT, DH], BF, tag="v")
        outT_sb = [consts.tile([128, HPC, 512], BF, tag=f"outT{c}", name=f"outT{c}")
                   for c in range(NCH)]
        xT_sb = consts.tile([128, KT, S], BF, tag="xT")
        cos_sb = consts.tile([128, S], BF, tag="cos")
        sin_sb = consts.tile([128, S], BF, tag="sin")
        perm_sb = consts.tile([128, 128], BF, tag="perm")
        ident_sb = consts.tile([128, 128], F32, tag="ident")

        # ---- input DMAs: 2D-sliced (3D strided views explode the
        # sequencer's descriptor-gen cost), consumption order, spread over
        # three triggering sequencers. wk/wv + small consts first so the
        # kk-outer kv-proj loop starts the moment the first xT tiles land;
        # wq/cos/sin/wo follow (needed progressively later).
        _dma_engines = [nc.sync, nc.gpsimd, nc.scalar]
        _dma_i = [0]

        def dma_in(out, in_):
            eng = _dma_engines[_dma_i[0] % len(_dma_engines)]
            _dma_i[0] += 1
            eng.dma_start(out=out, in_=in_)

        dma_in(wk_sb[:, 0, :], wk_d[0:128, :])
        dma_in(wv_sb[:, 0, :], wv_d[0:128, :])
        for kk in range(KT):
            dma_in(xT_sb[:, kk, :], xT_d[kk * 128:(kk + 1) * 128, :])
            if kk + 1 < KT:
                dma_in(wk_sb[:, kk + 1, :],
                       wk_d[(kk + 1) * 128:(kk + 2) * 128, :])
                dma_in(wv_sb[:, kk + 1, :],
                       wv_d[(kk + 1) * 128:(kk + 2) * 128, :])
            if kk == 0:
                dma_in(perm_sb, perm_d)
                dma_in(mask_sb, mask_d)
                dma_in(bq_sb, bq_d)
                dma_in(bk_sb, bk_d)
                dma_in(bv_sb, bv_d)
        # late inputs ride only sync/gpsimd: the scalar sequencer must be
        # free for the projection evictions the moment the kv loop ends
        _late = [nc.sync, nc.gpsimd]
        _late[0].dma_start(out=cos_sb, in_=cos_d)
        _late[1].dma_start(out=sin_sb, in_=sin_d)
        for kk in range(KT):
            _late[kk % 2].dma_start(
                out=wq_sb[:, kk, :], in_=wq_d[kk * 128:(kk + 1) * 128, :])
        for h in range(HPC):
            _late[h % 2].dma_start(
                out=wo_sb[:, h, :], in_=wo_d[h * 128:(h + 1) * 128, :])
        nc.vector.memset(ones_sb, 1.0)
        make_identity(nc, ident_sb)

        def rope_store(raw, dst, bias_ap, c):
            # rot lives in poolO (never blocks the score-psum ring) and is
            # evicted to SBUF by ScalarE immediately, so the sin-multiply
            # runs in DVE's 2x bf16 mode instead of 1x f32-from-PSUM and
            # the bank frees right away.
            rot = poolO.tile([128, 512], F32, tag="o512", name="rot")
            nc.tensor.matmul(rot, perm_sb, raw, start=True, stop=True)
            rsb = tmpp.tile([128, 512], BF, tag="rsb", name="rsb")
            nc.scalar.copy(rsb, rot)
            t1 = tmpp.tile([128, 512], BF, tag="t1", name="t1")
            nc.vector.tensor_mul(t1, raw, cos_sb[:, c * 512:(c + 1) * 512])
            t2 = tmpp.tile([128, 512], BF, tag="t2", name="t2")
            nc.vector.tensor_mul(t2, rsb, sin_sb[:, c * 512:(c + 1) * 512])
            # dst = (t2 + bias) + t1
            nc.vector.scalar_tensor_tensor(dst, t2, bias_ap, t1,
                                           op0=ALU.add, op1=ALU.add)

        # ---- ramp: k and v projections together, kk-outer (8 concurrent
        # PSUM accumulators) so PE density tracks the xT DMA stream.
        pss_k = [poolM.tile([128, 512], F32, tag="m512", name=f"kps{_c}")
                 for _c in range(NCH)]
        pss_v = [poolM.tile([128, 512], F32, tag="m512", name=f"vps{_c}")
                 for _c in range(2)]
        pss_v += [poolO.tile([128, 512], F32, tag="o512", name=f"vps{_c}")
                  for _c in range(2, NCH)]
        for kk in range(KT):
            for c in range(NCH):
                nc.tensor.matmul(pss_k[c], wk_sb[:, kk, :],
                                 xT_sb[:, kk, c * 512:(c + 1) * 512],
                                 start=(kk == 0), stop=(kk == KT - 1))
            for c in range(NCH):
                nc.tensor.matmul(pss_v[c], wv_sb[:, kk, :],
                                 xT_sb[:, kk, c * 512:(c + 1) * 512],
                                 start=(kk == 0), stop=(kk == KT - 1))
        for c in range(NCH):
            raw = rawp.tile([128, 512], BF, tag="kraw", name="kraw")
            nc.scalar.copy(raw, pss_k[c])
            rope_store(raw, kT_sb[:, c * 512:(c + 1) * 512], bk_sb[:, 0:1], c)
            vraw = rawp.tile([128, 512], F32, tag="vraw", name="vraw")
            nc.scalar.activation(vraw, pss_v[c], func=AF.Identity,
                                 bias=bv_sb[:, 0:1])
            for j in range(4):
                t = c * 4 + j
                tp = poolM.tile([128, 128], F32, tag="m512", name="vtps")
                nc.tensor.transpose(tp, vraw[:, j * 128:(j + 1) * 128],
                                    ident_sb)
                nc.vector.tensor_copy(v_sb[:, t, :], tp)

        def emit_q_mms(h):
            pss = [poolM.tile([128, 512], F32, tag="m512", name=f"qps{_c}")
                   for _c in range(NCH)]
            for kk in range(KT):
                for c in range(NCH):
                    nc.tensor.matmul(pss[c],
                                     wq_sb[:, kk, h * 128:(h + 1) * 128],
                                     xT_sb[:, kk, c * 512:(c + 1) * 512],
                                     start=(kk == 0), stop=(kk == KT - 1))
            raws = []
            for c in range(NCH):
                raw = rawp.tile([128, 512], BF, tag="qraw", name="qraw",
                                bufs=8)
                nc.scalar.copy(raw, pss[c])
                raws.append(raw)
            return raws

        def emit_q_rope(h, raws):
            for c in range(NCH):
                rope_store(raws[c], qT_sb[h][:, c * 512:(c + 1) * 512],
                           bq_sb[:, h:h + 1], c)

        def emit_q_proj(h):
            emit_q_rope(h, emit_q_mms(h))

        def emit_c_group(m, np_, dve=True, ns=None, both_dve=False):
            if ns is None:
                ns = (2 * np_, 2 * np_ + 1)
            mc, mo = divmod(m, 4)
            pso = {n: poolM.tile([128, 512], F32, tag="m512", name=f"cpsum{n}")
                   for n in ns}
            for h in range(HPC):
                for n in ns:
                    nc.tensor.matmul(pso[n],
                                     outT_sb[mc][:, h, mo * 128:(mo + 1) * 128],
                                     wo_sb[:, h, n * 512:(n + 1) * 512],
                                     start=(h == 0), stop=(h == HPC - 1))
            ob = osbp.tile([128, 512 * len(ns)], BF, tag="osb", name="osb")
            for j, n in enumerate(ns):
                # split the PSUM evictions across both PSUM-capable engines
                if both_dve or (j % 2 == 0) == dve:
                    nc.vector.tensor_copy(ob[:, j * 512:(j + 1) * 512], pso[n])
                else:
                    nc.scalar.copy(ob[:, j * 512:(j + 1) * 512], pso[n])
            nc.gpsimd.dma_start(
                out=out_d[m * 128:(m + 1) * 128,
                          ns[0] * 512:(ns[-1] + 1) * 512],
                in_=ob)

        def emit_b_pair(c, hp, fill_ms=()):
            nt = 4 * c + 4
            hs = (2 * hp, 2 * hp + 1)
            out_ps = {h: poolO.tile([128, 512], F32, tag="o512",
                                    name=f"outps{h}") for h in hs}
            # bf16 ladder tree for the softmax denominators: quad partial
            # sums (depth <=3) combined pairwise at the end; stays in DVE's
            # 2x 16-bit mode and keeps rounding depth ~5 (<0.5% on sigma).
            lad = {h: [] for h in hs}
            cur = {h: None for h in hs}

            def emit_scores(t):
                jb = t - 4 * c
                off = max(jb, 0) * 128   # first valid sq column
                cl, ch_ = c * 512 + off, (c + 1) * 512
                es = {}
                for h in hs:
                    s_ps = poolM.tile([128, 512], F32, tag="m512", name="s_ps")
                    nc.tensor.matmul(s_ps[:, off:],
                                     kT_sb[:, t * 128:(t + 1) * 128],
                                     qT_sb[h][:, cl:ch_],
                                     start=True, stop=True)
                    e = expp.tile([128, 512], BF, tag="exp", name="e")
                    nc.scalar.activation(e[:, off:], s_ps[:, off:],
                                         func=AF.Exp, scale=SCALE)
                    if jb >= 0:
                        nc.vector.tensor_mul(e[:, off:off + 128],
                                             e[:, off:off + 128], mask_sb)
                    es[h] = e
                return es

            def emit_consume(t, es):
                jb = t - 4 * c
                off = max(jb, 0) * 128
                for h in hs:
                    nc.tensor.matmul(out_ps[h][:, off:], v_sb[:, t, :],
                                     es[h][:, off:],
                                     start=(t == 0), stop=(t == nt - 1))
                for h in hs:
                    e = es[h]
                    if jb < 0:          # full-width tile, quad position t%4
                        if t % 4 == 0:
                            q = ladp.tile([128, 512], BF, tag="lad",
                                          name="lad")
                            cur[h] = q
                            nc.vector.tensor_copy(q, e)
                        else:
                            nc.vector.tensor_add(cur[h], cur[h], e)
                            if t % 4 == 3:
                                lad[h].append(cur[h])
                                cur[h] = None
                    else:               # diagonal group (off grows with jb)
                        if jb == 0:
                            q = ladp.tile([128, 512], BF, tag="lad",
                                          name="lad")
                            cur[h] = q
                            nc.vector.tensor_copy(q, e)
                        else:
                            nc.vector.tensor_add(cur[h][:, off:],
                                                 cur[h][:, off:], e[:, off:])
                        if jb == 3:
                            lad[h].append(cur[h])
                            cur[h] = None

            # depth-2 software pipeline: scores for t+1 and t+2 are in flight
            # before the PV/ladder consumers of t, so PE never waits on exp
            pend = []
            for t in range(min(2, nt)):
                pend.append(emit_scores(t))
            for t in range(2, nt):
                emit_consume(t - 2, pend.pop(0))
                pend.append(emit_scores(t))
            for i, es in enumerate(pend):
                emit_consume(nt - len(pend) + i, es)

            # pairwise-combine the quad sums into one sigma ladder tile
            for h in hs:
                ts = lad[h]
                while len(ts) > 1:
                    nxt = []
                    for i in range(0, len(ts) - 1, 2):
                        nc.vector.tensor_add(ts[i], ts[i], ts[i + 1])
                        nxt.append(ts[i])
                    if len(ts) % 2:
                        nxt.append(ts[-1])
                    ts = nxt
                lad[h] = ts[0]

            # evict accumulators to SBUF with fast ACT copies so the PSUM
            # banks free quickly
            outU = {}
            for h in hs:
                u = outup.tile([128, 512], F32, tag="outU", name="outU")
                if c >= 2:
                    nc.vector.tensor_copy(u, out_ps[h])
                else:
                    nc.scalar.copy(u, out_ps[h])
                outU[h] = u

            # fill the boundary chain latency with out-proj work of the
            # previous chunk (its outT rows are complete)
            for m in fill_ms:
                for np_ in range(NCH // 2):
                    emit_c_group(m, np_, dve=(c >= 2), both_dve=(c >= 2))

            # sigma reduce+broadcast in one PE matmul against the ones
            # matrix (out[p,n] = sum_k lad[k,n] for every p), then the fast
            # reciprocal approximation and the fused normalize multiply.
            # Emitted after the fills so the in-order DVE stream never
            # bubbles waiting on cross-engine chains; outT[c] is only
            # needed a chunk boundary later.
            sgp = {}
            for h in hs:
                sg = poolO.tile([128, 512], F32, tag="o512", name="sgps")
                nc.tensor.matmul(sg, ones_sb, lad[h], start=True, stop=True)
                sgp[h] = sg
            for h in hs:
                rinv = sigp.tile([128, 512], F32, tag="sig", name="rinv")
                nc.vector.reciprocal_approx_fast(rinv, sgp[h])
                nc.vector.tensor_mul(outT_sb[c][:, h, :], outU[h], rinv)

        # ---- interleave: q-projections sandwich the first attention
        # chunk; each head's rope emits after the NEXT head's projection
        # block so the rot matmuls never wait on the ACT eviction.
        r0 = emit_q_mms(0)
        r1 = emit_q_mms(1)
        emit_q_rope(0, r0)
        emit_q_rope(1, r1)
        emit_b_pair(0, 0)
        r2 = emit_q_mms(2)
        r3 = emit_q_mms(3)
        emit_q_rope(2, r2)
        emit_q_rope(3, r3)
        emit_b_pair(0, 1)
        for c in range(1, NCH):
            for hp in range(HPC // 2):
                base = 4 * (c - 1) + 2 * hp
                emit_b_pair(c, hp, fill_ms=(base, base + 1))
        for m in range(4 * (NCH - 1), 4 * NCH):
            emit_c_group(m, 0, dve=(m % 2 == 1), ns=(0, 1))
            emit_c_group(m, 1, dve=(m % 2 == 0), ns=(2, 3))

    nc.compile()
    return nc


def _get_program():
    global _PROGRAM
    if _PROGRAM is None:
        _PROGRAM = _build_program()
    return _PROGRAM


def _host_tables():
    bf16 = ml_dtypes.bfloat16
    pos = np.arange(S, dtype=np.float32)[:, None]
    i = np.arange(DH // 2, dtype=np.float32)
    omega = np.exp((-2.0 * i / DH * np.log(np.float32(1_000_000.0))).astype(np.float32))
    ang = (pos * omega).astype(np.float32)
    sinT = np.ascontiguousarray(np.repeat(np.sin(ang), 2, axis=-1).T)
    cosT = np.ascontiguousarray(np.repeat(np.cos(ang), 2, axis=-1).T)
    P = np.zeros((DH, DH), np.float32)
    for ii in range(DH // 2):
        P[2 * ii, 2 * ii + 1] = -1.0
        P[2 * ii + 1, 2 * ii] = 1.0
    permT = np.ascontiguousarray(P.T).astype(bf16)
    maskLT = np.triu(np.ones((128, 128), np.float32)).astype(bf16)
    return cosT, sinT, permT, maskLT


def _install_ntff_hook():
    """Optional: register the axon NTFF profiling hook (missing antenv.axon_hooks
    shim) so run_bass_kernel_spmd(trace=True) can capture HW exec time."""
    import types
    try:
        import antenv
        if 'antenv.axon_hooks' not in sys.modules:
            mod = types.ModuleType('antenv.axon_hooks')
            _hook = [None]
            mod.set_axon_ntff_profile_hook = lambda h: _hook.__setitem__(0, h)
            mod.get_axon_ntff_profile_hook = lambda: _hook[0]
            sys.modules['antenv.axon_hooks'] = mod
            antenv.axon_hooks = mod
        if '/root/.axon_site' not in sys.path:
            sys.path.insert(0, '/root/.axon_site')
        from trn_agent_boot.trn_boot import _ntff_profile_via_ctypes
        sys.modules['antenv.axon_hooks'].set_axon_ntff_profile_hook(
            _ntff_profile_via_ctypes('/opt/axon/libaxon_pjrt.so'))
        bass_utils.upload_artifacts = lambda tmpdir: tmpdir
        return True
    except Exception:
        return False


def kernel(x, wq, bq, wk, bk, wv, bv, wo, bo, masked=None, **_unused):
    global LAST_RESULTS
    bf16 = ml_dtypes.bfloat16
    nc = _get_program()

    x = np.asarray(x, np.float32)
    wq = np.asarray(wq, np.float32)
    wk = np.asarray(wk, np.float32)
    wv = np.asarray(wv, np.float32)
    wo = np.asarray(wo, np.float32)
    bq = np.asarray(bq, np.float32)
    bk = np.asarray(bk, np.float32)
    bv = np.asarray(bv, np.float32)
    bo = np.asarray(bo, np.float32)

    cosT, sinT, permT, maskLT = _host_tables()

    xT = [np.ascontiguousarray(x[b].T).astype(bf16) for b in range(B)]
    in_maps = []
    for core in range(N_CORES):
        b, g = divmod(core, G)
        cs = slice(g * 512, (g + 1) * 512)          # q-channel / out-channel slice
        ks = slice(g * 128, (g + 1) * 128)          # kv-channel slice
        in_maps.append({
            "xT": xT[b],
            "wqT": np.ascontiguousarray(wq[cs, :].T).astype(bf16),
            "wkT": np.ascontiguousarray(wk[ks, :].T).astype(bf16),
            "wvT": np.ascontiguousarray(wv[ks, :].T).astype(bf16),
            "woT": np.ascontiguousarray(wo[:, cs].T).astype(bf16),
            "cosT": cosT.astype(bf16),
            "sinT": sinT.astype(bf16),
            "permT": permT,
            "maskLT": maskLT,
            "bq": np.ascontiguousarray(bq[cs].reshape(HPC, DH).T),
            "bk": np.ascontiguousarray(bk[ks].reshape(DH, 1)),
            "bv": np.ascontiguousarray(bv[ks].reshape(DH, 1)),
        })

    trace = bool(TRACE)
    if trace:
        trace = _install_ntff_hook()
    res = bass_utils.run_bass_kernel_spmd(nc, in_maps,
                                          core_ids=list(range(N_CORES)),
                                          trace=trace)
    LAST_RESULTS = res

    out = np.zeros((B, S, D), np.float32)
    for core in range(N_CORES):
        b = core // G
        out[b] += np.asarray(res.results[core]["part"], np.float32)
    out += bo[None, None, :]
    return out


# revision 32
# speedup vs baseline: 1.1727x; 1.0184x over previous
"""Trainium2 Bass kernel for GQA attention (B=2, S=2048, D=2048, H=16, G=4 kv-heads,
DH=128) with interleaved RoPE (base 1e6) and causal mask.

Sharding: one (batch b, kv-group g) pair per NeuronCore -> 8 cores. Each core
computes its 4 q-heads against its single kv-head (Megatron-style column-split
of w_q/w_k/w_v, row-split of w_o) and produces a partial (S, D) output-projection
product in bf16; the host sums the 4 partials per batch and adds bo.

Device dataflow per core (all matmuls bf16 with f32 PSUM accumulate):
  A) qT/kT/vT = W^T-slices @ x^T (transposed projections, dmodel contraction),
     interleaved RoPE applied in the transposed layout via a +-1 permutation
     matmul plus two DVE multiplies with host-provided cos/sin tables;
     v transposed back to [sk, dh] via PE transpose. Inputs arrive via a
     handful of batched rearranged DMAs (xT in 4 groups that the kk-outer
     kv-projection loop streams behind).
  B) per (head, sq-chunk of 512): scoresT tiles [sk=128, sq=512] via PE with a
     depth-2 software pipeline, exp on ScalarE (scale=1/sqrt(128)) straight out
     of PSUM -> bf16 attn weights, causal masking on diagonal tiles, PV matmul
     accumulates out^T [dh, sq] in PSUM over sk tiles. Softmax denominators
     accumulate on DVE as a shallow bf16 ladder tree (2x mode); the partition
     reduction AND broadcast happen in a single PE matmul against a 128x128
     ones matrix, inverted with the fast DVE reciprocal approximation (emitted
     after the fill work so the in-order DVE stream never bubbles), then a
     fused normalize multiply.
  C) partial = out_heads^T^T @ wo^T-slice; PSUM evictions alternate between
     ScalarE and DVE (both are loaded inside attention chunks), batched bf16
     DMAs out.
"""
import sys
import os

if '/opt/trn_rl_repo' not in sys.path:
    sys.path.insert(0, '/opt/trn_rl_repo')

import numpy as np
import ml_dtypes

from contextlib import ExitStack

import concourse.bass as bass
import concourse.bass_isa as bass_isa
import concourse.mybir as mybir
import concourse.tile as tile
from concourse import bacc
import concourse.bass_utils as bass_utils
from concourse.masks import make_identity

BF = mybir.dt.bfloat16
F32 = mybir.dt.float32
AF = mybir.ActivationFunctionType
ALU = mybir.AluOpType
RED = bass_isa.ReduceOp

B, S, D, H, G = 2, 2048, 2048, 16, 4
DH = 128
HPC = H // G          # q heads per core
KT = D // 128         # dmodel k-tiles
NCH = S // 512        # sq chunks
SCALE = float(1.0 / np.sqrt(DH))
N_CORES = 8

TRACE = False          # set by test harness to capture an NTFF profile
LAST_RESULTS = None    # BassKernelResults of the most recent run (for test.py)

_PROGRAM = None


def _build_program():
    nc = bacc.Bacc("TRN2", target_bir_lowering=False, debug=False,
                   num_devices=N_CORES)

    def din(name, shape, dtype=BF):
        return nc.dram_tensor(name, shape, dtype, kind="ExternalInput").ap()

    xT_d = din("xT", [D, S])
    wq_d = din("wqT", [D, 512])
    wk_d = din("wkT", [D, DH])
    wv_d = din("wvT", [D, DH])
    wo_d = din("woT", [512, D])
    cos_d = din("cosT", [DH, S])
    sin_d = din("sinT", [DH, S])
    perm_d = din("permT", [DH, DH])
    mask_d = din("maskLT", [DH, DH])
    bq_d = din("bq", [DH, HPC], F32)
    bk_d = din("bk", [DH, 1], F32)
    bv_d = din("bv", [DH, 1], F32)
    out_d = nc.dram_tensor("part", [S, D], BF, kind="ExternalOutput").ap()

    with tile.TileContext(nc) as tc, ExitStack() as ctx:
        consts = ctx.enter_context(tc.tile_pool(name="consts", bufs=1))
        # PSUM: poolM rotates 6 banks among kv/q-proj, rot, vtrans, the
        # depth-2 score pipeline and out-proj psums; poolO rotates 2 among
        # the ramp v-proj tails and the attention out accumulators.
        poolM = ctx.enter_context(tc.tile_pool(name="poolM", bufs=6, space="PSUM"))
        poolO = ctx.enter_context(tc.tile_pool(name="poolO", bufs=2, space="PSUM"))
        rawp = ctx.enter_context(tc.tile_pool(name="rawp", bufs=2))
        tmpp = ctx.enter_context(tc.tile_pool(name="tmpp", bufs=2))
        expp = ctx.enter_context(tc.tile_pool(name="expp", bufs=9))
        ladp = ctx.enter_context(tc.tile_pool(name="ladp", bufs=9))
        sigp = ctx.enter_context(tc.tile_pool(name="sigp", bufs=2))
        osbp = ctx.enter_context(tc.tile_pool(name="osbp", bufs=3))
        outup = ctx.enter_context(tc.tile_pool(name="outup", bufs=3))

        # persistent SBUF tensors
        wq_sb = consts.tile([128, KT, 512], BF, tag="wq")
        wk_sb = consts.tile([128, KT, DH], BF, tag="wk")
        wv_sb = consts.tile([128, KT, DH], BF, tag="wv")
        wo_sb = consts.tile([128, HPC, D], BF, tag="wo")
        mask_sb = consts.tile([128, 128], BF, tag="mask")
        bq_sb = consts.tile([128, HPC], F32, tag="bq")
        bk_sb = consts.tile([128, 1], F32, tag="bk")
        bv_sb = consts.tile([128, 1], F32, tag="bv")
        ones_sb = consts.tile([128, 128], BF, tag="ones")
        qT_sb = [consts.tile([128, S], BF, tag=f"qT{h}", name=f"qT{h}")
                 for h in range(HPC)]
        kT_sb = consts.tile([128, S], BF, tag="kT")
        v_sb = consts.tile([128, KT, DH], BF, tag="v")
        outT_sb = [consts.tile([128, HPC, 512], BF, tag=f"outT{c}", name=f"outT{c}")
                   for c in range(NCH)]
        xT_sb = consts.tile([128, KT, S], BF, tag="xT")
        cos_sb = consts.tile([128, S], BF, tag="cos")
        sin_sb = consts.tile([128, S], BF, tag="sin")
        perm_sb = consts.tile([128, 128], BF, tag="perm")
        ident_sb = consts.tile([128, 128], F32, tag="ident")

        # ---- input DMAs: 2D-sliced (3D strided views explode the
        # sequencer's descriptor-gen cost), consumption order, spread over
        # three triggering sequencers. wk/wv + small consts first so the
        # kk-outer kv-proj loop starts the moment the first xT tiles land;
        # wq/cos/sin/wo follow (needed progressively later).
        _dma_engines = [nc.sync, nc.gpsimd, nc.scalar]
        _dma_i = [0]

        def dma_in(out, in_):
            eng = _dma_engines[_dma_i[0] % len(_dma_engines)]
            _dma_i[0] += 1
            eng.dma_start(out=out, in_=in_)

        dma_in(wk_sb[:, 0, :], wk_d[0:128, :])
        dma_in(wv_sb[:, 0, :], wv_d[0:128, :])
        for kk in range(KT):
            dma_in(xT_sb[:, kk, :], xT_d[kk * 128:(kk + 1) * 128, :])
            if kk + 1 < KT:
                dma_in(wk_sb[:, kk + 1, :],
                       wk_d[(kk + 1) * 128:(kk + 2) * 128, :])
                dma_in(wv_sb[:, kk + 1, :],
                       wv_d[(kk + 1) * 128:(kk + 2) * 128, :])
            if kk == 0:
                dma_in(perm_sb, perm_d)
                dma_in(mask_sb, mask_d)
                dma_in(bq_sb, bq_d)
                dma_in(bk_sb, bk_d)
                dma_in(bv_sb, bv_d)
        # late inputs ride only sync/gpsimd: the scalar sequencer must be
        # free for the projection evictions the moment the kv loop ends
        _late = [nc.sync, nc.gpsimd]
        _late[0].dma_start(out=cos_sb, in_=cos_d)
        _late[1].dma_start(out=sin_sb, in_=sin_d)
        for kk in range(KT):
            _late[kk % 2].dma_start(
                out=wq_sb[:, kk, :], in_=wq_d[kk * 128:(kk + 1) * 128, :])
        for h in range(HPC):
            _late[h % 2].dma_start(
                out=wo_sb[:, h, :], in_=wo_d[h * 128:(h + 1) * 128, :])
        nc.vector.memset(ones_sb, 1.0)
        make_identity(nc, ident_sb)

        def rope_store(raw, dst, bias_ap, c):
            # rot lives in poolO (never blocks the score-psum ring) and is
            # evicted to SBUF by ScalarE immediately, so the sin-multiply
            # runs in DVE's 2x bf16 mode instead of 1x f32-from-PSUM and
            # the bank frees right away.
            rot = poolO.tile([128, 512], F32, tag="o512", name="rot")
            nc.tensor.matmul(rot, perm_sb, raw, start=True, stop=True)
            rsb = tmpp.tile([128, 512], BF, tag="rsb", name="rsb")
            nc.scalar.copy(rsb, rot)
            t1 = tmpp.tile([128, 512], BF, tag="t1", name="t1")
            nc.vector.tensor_mul(t1, raw, cos_sb[:, c * 512:(c + 1) * 512])
            t2 = tmpp.tile([128, 512], BF, tag="t2", name="t2")
            nc.vector.tensor_mul(t2, rsb, sin_sb[:, c * 512:(c + 1) * 512])
            # dst = (t2 + bias) + t1
            nc.vector.scalar_tensor_tensor(dst, t2, bias_ap, t1,
                                           op0=ALU.add, op1=ALU.add)

        # ---- ramp: k and v projections together, kk-outer (8 concurrent
        # PSUM accumulators) so PE density tracks the xT DMA stream.
        pss_k = [poolM.tile([128, 512], F32, tag="m512", name=f"kps{_c}")
                 for _c in range(NCH)]
        pss_v = [poolM.tile([128, 512], F32, tag="m512", name=f"vps{_c}")
                 for _c in range(2)]
        pss_v += [poolO.tile([128, 512], F32, tag="o512", name=f"vps{_c}")
                  for _c in range(2, NCH)]
        for kk in range(KT):
            for c in range(NCH):
                nc.tensor.matmul(pss_k[c], wk_sb[:, kk, :],
                                 xT_sb[:, kk, c * 512:(c + 1) * 512],
                                 start=(kk == 0), stop=(kk == KT - 1))
            for c in range(NCH):
                nc.tensor.matmul(pss_v[c], wv_sb[:, kk, :],
                                 xT_sb[:, kk, c * 512:(c + 1) * 512],
                                 start=(kk == 0), stop=(kk == KT - 1))
        for c in range(NCH):
            raw = rawp.tile([128, 512], BF, tag="kraw", name="kraw")
            nc.scalar.copy(raw, pss_k[c])
            rope_store(raw, kT_sb[:, c * 512:(c + 1) * 512], bk_sb[:, 0:1], c)
            vraw = rawp.tile([128, 512], F32, tag="vraw", name="vraw")
            nc.scalar.activation(vraw, pss_v[c], func=AF.Identity,
                                 bias=bv_sb[:, 0:1])
            for j in range(4):
                t = c * 4 + j
                tp = poolM.tile([128, 128], F32, tag="m512", name="vtps")
                nc.tensor.transpose(tp, vraw[:, j * 128:(j + 1) * 128],
                                    ident_sb)
                nc.vector.tensor_copy(v_sb[:, t, :], tp)

        def emit_q_mms(h):
            pss = [poolM.tile([128, 512], F32, tag="m512", name=f"qps{_c}")
                   for _c in range(NCH)]
            for kk in range(KT):
                for c in range(NCH):
                    nc.tensor.matmul(pss[c],
                                     wq_sb[:, kk, h * 128:(h + 1) * 128],
                                     xT_sb[:, kk, c * 512:(c + 1) * 512],
                                     start=(kk == 0), stop=(kk == KT - 1))
            raws = []
            for c in range(NCH):
                raw = rawp.tile([128, 512], BF, tag="qraw", name="qraw",
                                bufs=8)
                nc.scalar.copy(raw, pss[c])
                raws.append(raw)
            return raws

        def emit_q_rope(h, raws):
            for c in range(NCH):
                rope_store(raws[c], qT_sb[h][:, c * 512:(c + 1) * 512],
                           bq_sb[:, h:h + 1], c)

        def emit_q_rope_pair(hpair, raws_pair):
            # c-major across the head pair: both heads' chunk-0 qT columns
            # land first, so the next b_pair's first scores never wait on
            # the tail of the rope DVE chain
            for c in range(NCH):
                for h, raws in zip(hpair, raws_pair):
                    rope_store(raws[c], qT_sb[h][:, c * 512:(c + 1) * 512],
                               bq_sb[:, h:h + 1], c)

        def emit_q_proj(h):
            emit_q_rope(h, emit_q_mms(h))

        def emit_c_group(m, np_, dve=True, ns=None, both_dve=False):
            if ns is None:
                ns = (2 * np_, 2 * np_ + 1)
            mc, mo = divmod(m, 4)
            pso = {n: poolM.tile([128, 512], F32, tag="m512", name=f"cpsum{n}")
                   for n in ns}
            for h in range(HPC):
                for n in ns:
                    nc.tensor.matmul(pso[n],
                                     outT_sb[mc][:, h, mo * 128:(mo + 1) * 128],
                                     wo_sb[:, h, n * 512:(n + 1) * 512],
                                     start=(h == 0), stop=(h == HPC - 1))
            ob = osbp.tile([128, 512 * len(ns)], BF, tag="osb", name="osb")
            for j, n in enumerate(ns):
                # split the PSUM evictions across both PSUM-capable engines
                if both_dve or (j % 2 == 0) == dve:
                    nc.vector.tensor_copy(ob[:, j * 512:(j + 1) * 512], pso[n])
                else:
                    nc.scalar.copy(ob[:, j * 512:(j + 1) * 512], pso[n])
            nc.gpsimd.dma_start(
                out=out_d[m * 128:(m + 1) * 128,
                          ns[0] * 512:(ns[-1] + 1) * 512],
                in_=ob)

        def emit_b_pair(c, hp, fill_ms=()):
            nt = 4 * c + 4
            hs = (2 * hp, 2 * hp + 1)
            out_ps = {h: poolO.tile([128, 512], F32, tag="o512",
                                    name=f"outps{h}") for h in hs}
            # bf16 ladder tree for the softmax denominators: quad partial
            # sums (depth <=3) combined pairwise at the end; stays in DVE's
            # 2x 16-bit mode and keeps rounding depth ~5 (<0.5% on sigma).
            lad = {h: [] for h in hs}
            cur = {h: None for h in hs}

            def emit_scores(t):
                jb = t - 4 * c
                off = max(jb, 0) * 128   # first valid sq column
                cl, ch_ = c * 512 + off, (c + 1) * 512
                es = {}
                for h in hs:
                    s_ps = poolM.tile([128, 512], F32, tag="m512", name="s_ps")
                    nc.tensor.matmul(s_ps[:, off:],
                                     kT_sb[:, t * 128:(t + 1) * 128],
                                     qT_sb[h][:, cl:ch_],
                                     start=True, stop=True)
                    e = expp.tile([128, 512], BF, tag="exp", name="e")
                    nc.scalar.activation(e[:, off:], s_ps[:, off:],
                                         func=AF.Exp, scale=SCALE)
                    if jb >= 0:
                        nc.vector.tensor_mul(e[:, off:off + 128],
                                             e[:, off:off + 128], mask_sb)
                    es[h] = e
                return es

            def emit_consume(t, es):
                jb = t - 4 * c
                off = max(jb, 0) * 128
                for h in hs:
                    nc.tensor.matmul(out_ps[h][:, off:], v_sb[:, t, :],
                                     es[h][:, off:],
                                     start=(t == 0), stop=(t == nt - 1))
                for h in hs:
                    e = es[h]
                    if jb < 0:          # full-width tile, quad position t%4
                        if t % 4 == 0:
                            q = ladp.tile([128, 512], BF, tag="lad",
                                          name="lad")
                            cur[h] = q
                            nc.vector.tensor_copy(q, e)
                        else:
                            nc.vector.tensor_add(cur[h], cur[h], e)
                            if t % 4 == 3:
                                lad[h].append(cur[h])
                                cur[h] = None
                    else:               # diagonal group (off grows with jb)
                        if jb == 0:
                            q = ladp.tile([128, 512], BF, tag="lad",
                                          name="lad")
                            cur[h] = q
                            nc.vector.tensor_copy(q, e)
                        else:
                            nc.vector.tensor_add(cur[h][:, off:],
                                                 cur[h][:, off:], e[:, off:])
                        if jb == 3:
                            lad[h].append(cur[h])
                            cur[h] = None

            # depth-2 software pipeline: scores for t+1 and t+2 are in flight
            # before the PV/ladder consumers of t, so PE never waits on exp
            pend = []
            for t in range(min(2, nt)):
                pend.append(emit_scores(t))
            for t in range(2, nt):
                emit_consume(t - 2, pend.pop(0))
                pend.append(emit_scores(t))
            for i, es in enumerate(pend):
                emit_consume(nt - len(pend) + i, es)

            # pairwise-combine the quad sums into one sigma ladder tile
            for h in hs:
                ts = lad[h]
                while len(ts) > 1:
                    nxt = []
                    for i in range(0, len(ts) - 1, 2):
                        nc.vector.tensor_add(ts[i], ts[i], ts[i + 1])
                        nxt.append(ts[i])
                    if len(ts) % 2:
                        nxt.append(ts[-1])
                    ts = nxt
                lad[h] = ts[0]

            # evict accumulators to SBUF with fast ACT copies so the PSUM
            # banks free quickly
            outU = {}
            for h in hs:
                u = outup.tile([128, 512], F32, tag="outU", name="outU")
                if c >= 2:
                    nc.vector.tensor_copy(u, out_ps[h])
                else:
                    nc.scalar.copy(u, out_ps[h])
                outU[h] = u

            # fill the boundary chain latency with out-proj work of the
            # previous chunk (its outT rows are complete)
            for m in fill_ms:
                for np_ in range(NCH // 2):
                    emit_c_group(m, np_, dve=(c >= 2), both_dve=(c >= 2))

            # sigma reduce+broadcast in one PE matmul against the ones
            # matrix (out[p,n] = sum_k lad[k,n] for every p), then the fast
            # reciprocal approximation and the fused normalize multiply.
            # Emitted after the fills so the in-order DVE stream never
            # bubbles waiting on cross-engine chains; outT[c] is only
            # needed a chunk boundary later.
            sgp = {}
            for h in hs:
                sg = poolO.tile([128, 512], F32, tag="o512", name="sgps")
                nc.tensor.matmul(sg, ones_sb, lad[h], start=True, stop=True)
                sgp[h] = sg
            for h in hs:
                rinv = sigp.tile([128, 512], F32, tag="sig", name="rinv")
                nc.vector.reciprocal_approx_fast(rinv, sgp[h])
                nc.vector.tensor_mul(outT_sb[c][:, h, :], outU[h], rinv)

        # ---- interleave: q-projections sandwich the first attention
        # chunk; each head's rope emits after the NEXT head's projection
        # block so the rot matmuls never wait on the ACT eviction.
        r0 = emit_q_mms(0)
        r1 = emit_q_mms(1)
        emit_q_rope_pair((0, 1), (r0, r1))
        emit_b_pair(0, 0)
        r2 = emit_q_mms(2)
        r3 = emit_q_mms(3)
        emit_q_rope_pair((2, 3), (r2, r3))
        emit_b_pair(0, 1)
        for c in range(1, NCH):
            for hp in range(HPC // 2):
                base = 4 * (c - 1) + 2 * hp
                emit_b_pair(c, hp, fill_ms=(base, base + 1))
        for m in range(4 * (NCH - 1), 4 * NCH):
            emit_c_group(m, 0, dve=(m % 2 == 1), ns=(0, 1))
            emit_c_group(m, 1, dve=(m % 2 == 0), ns=(2, 3))

    nc.compile()
    return nc


def _get_program():
    global _PROGRAM
    if _PROGRAM is None:
        _PROGRAM = _build_program()
    return _PROGRAM


def _host_tables():
    bf16 = ml_dtypes.bfloat16
    pos = np.arange(S, dtype=np.float32)[:, None]
    i = np.arange(DH // 2, dtype=np.float32)
    omega = np.exp((-2.0 * i / DH * np.log(np.float32(1_000_000.0))).astype(np.float32))
    ang = (pos * omega).astype(np.float32)
    sinT = np.ascontiguousarray(np.repeat(np.sin(ang), 2, axis=-1).T)
    cosT = np.ascontiguousarray(np.repeat(np.cos(ang), 2, axis=-1).T)
    P = np.zeros((DH, DH), np.float32)
    for ii in range(DH // 2):
        P[2 * ii, 2 * ii + 1] = -1.0
        P[2 * ii + 1, 2 * ii] = 1.0
    permT = np.ascontiguousarray(P.T).astype(bf16)
    maskLT = np.triu(np.ones((128, 128), np.float32)).astype(bf16)
    return cosT, sinT, permT, maskLT


def _install_ntff_hook():
    """Optional: register the axon NTFF profiling hook (missing antenv.axon_hooks
    shim) so run_bass_kernel_spmd(trace=True) can capture HW exec time."""
    import types
    try:
        import antenv
        if 'antenv.axon_hooks' not in sys.modules:
            mod = types.ModuleType('antenv.axon_hooks')
            _hook = [None]
            mod.set_axon_ntff_profile_hook = lambda h: _hook.__setitem__(0, h)
            mod.get_axon_ntff_profile_hook = lambda: _hook[0]
            sys.modules['antenv.axon_hooks'] = mod
            antenv.axon_hooks = mod
        if '/root/.axon_site' not in sys.path:
            sys.path.insert(0, '/root/.axon_site')
        from trn_agent_boot.trn_boot import _ntff_profile_via_ctypes
        sys.modules['antenv.axon_hooks'].set_axon_ntff_profile_hook(
            _ntff_profile_via_ctypes('/opt/axon/libaxon_pjrt.so'))
        bass_utils.upload_artifacts = lambda tmpdir: tmpdir
        return True
    except Exception:
        return False


def kernel(x, wq, bq, wk, bk, wv, bv, wo, bo, masked=None, **_unused):
    global LAST_RESULTS
    bf16 = ml_dtypes.bfloat16
    nc = _get_program()

    x = np.asarray(x, np.float32)
    wq = np.asarray(wq, np.float32)
    wk = np.asarray(wk, np.float32)
    wv = np.asarray(wv, np.float32)
    wo = np.asarray(wo, np.float32)
    bq = np.asarray(bq, np.float32)
    bk = np.asarray(bk, np.float32)
    bv = np.asarray(bv, np.float32)
    bo = np.asarray(bo, np.float32)

    cosT, sinT, permT, maskLT = _host_tables()

    xT = [np.ascontiguousarray(x[b].T).astype(bf16) for b in range(B)]
    in_maps = []
    for core in range(N_CORES):
        b, g = divmod(core, G)
        cs = slice(g * 512, (g + 1) * 512)          # q-channel / out-channel slice
        ks = slice(g * 128, (g + 1) * 128)          # kv-channel slice
        in_maps.append({
            "xT": xT[b],
            "wqT": np.ascontiguousarray(wq[cs, :].T).astype(bf16),
            "wkT": np.ascontiguousarray(wk[ks, :].T).astype(bf16),
            "wvT": np.ascontiguousarray(wv[ks, :].T).astype(bf16),
            "woT": np.ascontiguousarray(wo[:, cs].T).astype(bf16),
            "cosT": cosT.astype(bf16),
            "sinT": sinT.astype(bf16),
            "permT": permT,
            "maskLT": maskLT,
            "bq": np.ascontiguousarray(bq[cs].reshape(HPC, DH).T),
            "bk": np.ascontiguousarray(bk[ks].reshape(DH, 1)),
            "bv": np.ascontiguousarray(bv[ks].reshape(DH, 1)),
        })

    trace = bool(TRACE)
    if trace:
        trace = _install_ntff_hook()
    res = bass_utils.run_bass_kernel_spmd(nc, in_maps,
                                          core_ids=list(range(N_CORES)),
                                          trace=trace)
    LAST_RESULTS = res

    out = np.zeros((B, S, D), np.float32)
    for core in range(N_CORES):
        b = core // G
        out[b] += np.asarray(res.results[core]["part"], np.float32)
    out += bo[None, None, :]
    return out
